# revision 1
# baseline (speedup 1.0000x reference)
"""Trainium2 Bass kernel: batched single-head attention + gate MLP.

Per-core (data-parallel over batch, 1 batch row per core):
  q = query @ Wq.T + bq ; k,v likewise
  scores = q @ k.T / sqrt(768); attn = softmax(scores)
  attended = attn @ v
  h = relu(attended @ Wg1.T + bg1); gate = sigmoid(h @ Wg2.T + bg2)
  out = sigmoid(gate) * attended * text_scale

Matmuls run in float32r (TF32-like, 12-bit mantissa) at full PE rate.
All contractions put the reduced dim on partitions, so the three input
tensors and the five weights are transposed on the PE via identity
matmuls. qT is bounced through DRAM to fit SBUF.
"""
import numpy as np

import concourse.bass as bass
import concourse.mybir as mybir
import concourse.tile as tile
from concourse import bacc
from concourse.bass_utils import run_bass_kernel_spmd

F32 = mybir.dt.float32
F32R = mybir.dt.float32r
AF = mybir.ActivationFunctionType

B, S, D = 8, 2048, 768
EB = D // 128           # 6 blocks of the feature dim
SB = S // 128           # 16 blocks of the seq dim
PCH = 512               # projection s-chunk
NPCH = S // PCH         # 4
ICH = 256               # attention/gate i-chunk
NICH = S // ICH         # 8
SCALE = 1.0 / float(np.sqrt(D))

_CACHE = {}


def _build(reps=1):
    nc = bacc.Bacc(None)

    query = nc.dram_tensor("query", [S, D], F32, kind="ExternalInput")
    key = nc.dram_tensor("key", [S, D], F32, kind="ExternalInput")
    value = nc.dram_tensor("value", [S, D], F32, kind="ExternalInput")
    Wq = nc.dram_tensor("Wq", [D, D], F32, kind="ExternalInput")
    Wk = nc.dram_tensor("Wk", [D, D], F32, kind="ExternalInput")
    Wv = nc.dram_tensor("Wv", [D, D], F32, kind="ExternalInput")
    Wg1 = nc.dram_tensor("Wg1", [D, D], F32, kind="ExternalInput")
    Wg2 = nc.dram_tensor("Wg2", [D, D], F32, kind="ExternalInput")
    bq = nc.dram_tensor("bq", [D], F32, kind="ExternalInput")
    bk = nc.dram_tensor("bk", [D], F32, kind="ExternalInput")
    bv = nc.dram_tensor("bv", [D], F32, kind="ExternalInput")
    bg1 = nc.dram_tensor("bg1", [D], F32, kind="ExternalInput")
    bg2 = nc.dram_tensor("bg2", [D], F32, kind="ExternalInput")
    ts = nc.dram_tensor("ts", [1, D], F32, kind="ExternalInput")
    ident = nc.dram_tensor("ident", [128, 128], F32, kind="ExternalInput")
    ones = nc.dram_tensor("ones", [128, 128], F32, kind="ExternalInput")
    out = nc.dram_tensor("out", [S, D], F32, kind="ExternalOutput")

    with tile.TileContext(nc) as tc:
        with tc.tile_pool(name="persist", bufs=1) as P, \
             tc.tile_pool(name="psc", bufs=2, space="PSUM") as PSC, \
             tc.tile_pool(name="pmm", bufs=2, space="PSUM") as PMM, \
             tc.tile_pool(name="dram", bufs=1, space="DRAM") as DR:

            ident_sb = P.tile([128, 128], F32R, tag="ident")
            nc.gpsimd.dma_start(out=ident_sb, in_=ident[:, :])
            ones_sb = P.tile([128, 128], F32R, tag="ones")
            nc.gpsimd.dma_start(out=ones_sb, in_=ones[:, :])

            kT = P.tile([128, EB, S], F32R, tag="kT")        # k^T [e, s]
            v_sb = P.tile([128, SB, D], F32R, tag="v")       # v [j, e]

            def vec_sb(name, src):                           # [D] -> [128, EB]
                t = P.tile([128, EB], F32, tag=name)
                nc.sync.dma_start(out=t, in_=src.rearrange("(b p) -> p b", p=128))
                return t

            bq_sb = vec_sb("bq", bq[:])
            bk_sb = vec_sb("bk", bk[:])
            bg1_sb = vec_sb("bg1", bg1[:])
            bg2_sb = vec_sb("bg2", bg2[:])
            ts_sb = vec_sb("ts", ts[0, :])
            bg2h_sb = P.tile([128, EB], F32, tag="bg2h")
            nc.vector.tensor_scalar_mul(bg2h_sb, bg2_sb, 0.5)
            tsh_sb = P.tile([128, EB], F32, tag="tsh")
            nc.vector.tensor_scalar_mul(tsh_sb, ts_sb, 0.5)

            qT_dram = DR.tile([D, S], F32R, tag="qTdram")

            def load_wT(wdram, wT, pool):
                """DMA W [e,d] fp32, transpose on PE, cast to f32r on evict."""
                wst = pool.tile([128, EB, D], F32R, tag="wstage", bufs=1)
                nc.gpsimd.dma_start(
                    out=wst, in_=wdram.rearrange("(eb p) d -> p eb d", p=128))
                for db in range(EB):
                    for eb0 in range(0, EB, 3):
                        tp = PSC.tile([128, 384], F32R, tag="sc")
                        for k in range(3):
                            nc.tensor.transpose(
                                tp[:, k * 128:(k + 1) * 128],
                                wst[:, eb0 + k, db * 128:(db + 1) * 128], ident_sb)
                        nc.vector.tensor_copy(
                            wT[:, db, eb0 * 128:(eb0 + 3) * 128], tp)

            def load_xT(xdram, c, pool, tag):
                """DMA input s-chunk c (cast->f32r) + transpose -> [p, db, s]."""
                nsb = PCH // 128
                xst = pool.tile([128, nsb, D], F32R, tag=tag + "st", bufs=2)
                nc.gpsimd.dma_start(
                    out=xst,
                    in_=xdram[c * PCH:(c + 1) * PCH, :].rearrange(
                        "(sb p) d -> p sb d", p=128))
                xT = pool.tile([128, EB, PCH], F32R, tag=tag + "T", bufs=1)
                for sb in range(nsb):
                    for db0 in range(0, EB, 3):
                        tp = PSC.tile([128, 3, 128], F32R, tag="sc")
                        for k in range(3):
                            nc.tensor.transpose(
                                tp[:, k, :],
                                xst[:, sb, (db0 + k) * 128:(db0 + k + 1) * 128],
                                ident_sb)
                        nc.vector.tensor_copy(
                            xT[:, db0:db0 + 3, sb * 128:(sb + 1) * 128], tp)
                return xT

            for _rep in range(reps):
                # ---- Phase A: project key -> kT, value -> v ----
                with tc.tile_pool(name="phA", bufs=2) as PA:
                    wkT = PA.tile([128, EB, D], F32R, tag="wkT", bufs=1)
                    bv_bc = PA.tile([128, D], F32, tag="bv", bufs=1)
                    nc.sync.dma_start(out=bv_bc, in_=bv[:].partition_broadcast(128))
                    wvT = PA.tile([128, EB, D], F32R, tag="wvT", bufs=1)
                    load_wT(Wk, wkT, PA)
                    load_wT(Wv, wvT, PA)
                    for c in range(NPCH):
                        kxT = load_xT(key, c, PA, "x")
                        for eb in range(EB):
                            ps = PSC.tile([128, PCH], F32, tag="sc")
                            for db in range(EB):
                                nc.tensor.matmul(
                                    ps, wkT[:, db, eb * 128:(eb + 1) * 128],
                                    kxT[:, db, :], start=(db == 0), stop=(db == EB - 1))
                            nc.scalar.activation(
                                kT[:, eb, c * PCH:(c + 1) * PCH], ps, AF.Identity,
                                bias=bk_sb[:, eb:eb + 1])
                        vxT = load_xT(value, c, PA, "x")
                        for jb in range(PCH // 128):
                            pv = PMM.tile([128, D], F32, tag="mm")
                            for n0, n1 in ((0, 512), (512, 768)):
                                for db in range(EB):
                                    nc.tensor.matmul(
                                        pv[:, n0:n1],
                                        vxT[:, db, jb * 128:(jb + 1) * 128],
                                        wvT[:, db, n0:n1],
                                        start=(db == 0), stop=(db == EB - 1))
                            nc.vector.tensor_add(
                                v_sb[:, c * (PCH // 128) + jb, :], pv[:, 0:D], bv_bc)

                # ---- Phase B: project query -> qT (DRAM bounce); load gate W ----
                persist2 = tc.tile_pool(name="persist2", bufs=1)
                P2 = persist2.__enter__()
                wg1T = P2.tile([128, EB, D], F32R, tag="wg1T")
                wg2T = P2.tile([128, EB, D], F32R, tag="wg2T")
                with tc.tile_pool(name="phB", bufs=2) as PB:
                    wqT = PB.tile([128, EB, D], F32R, tag="wqT", bufs=1)
                    load_wT(Wq, wqT, PB)
                    load_wT(Wg1, wg1T, PB)
                    load_wT(Wg2, wg2T, PB)
                    for c in range(NPCH):
                        qxT = load_xT(query, c, PB, "x")
                        for eb in range(EB):
                            ps = PSC.tile([128, PCH], F32, tag="sc")
                            for db in range(EB):
                                nc.tensor.matmul(
                                    ps, wqT[:, db, eb * 128:(eb + 1) * 128],
                                    qxT[:, db, :], start=(db == 0), stop=(db == EB - 1))
                            qrow = PB.tile([128, PCH], F32R, tag="qrow", bufs=1)
                            nc.scalar.activation(
                                qrow, ps, AF.Identity, bias=bq_sb[:, eb:eb + 1])
                            nc.sync.dma_start(
                                out=qT_dram[eb * 128:(eb + 1) * 128,
                                            c * PCH:(c + 1) * PCH],
                                in_=qrow)

                # ---- Phase C: attention + gate, i-chunks of ICH ----
                with tc.tile_pool(name="phC", bufs=2) as PC, \
                     tc.tile_pool(name="phC1", bufs=1) as PC1:
                    nib = ICH // 128
                    for ic in range(NICH):
                        qTc = PC.tile([128, EB, ICH], F32R, tag="qTc", bufs=1)
                        nc.sync.dma_start(
                            out=qTc,
                            in_=qT_dram[:, ic * ICH:(ic + 1) * ICH].rearrange(
                                "(eb p) i -> p eb i", p=128))
                        attnT = PC1.tile([128, SB, ICH], F32R, tag="attnT")
                        for jb in range(SB):
                            ps = PSC.tile([128, ICH], F32, tag="sc")
                            for eb in range(EB):
                                nc.tensor.matmul(
                                    ps, kT[:, eb, jb * 128:(jb + 1) * 128],
                                    qTc[:, eb, :],
                                    start=(eb == 0), stop=(eb == EB - 1))
                            nc.scalar.activation(
                                attnT[:, jb, :], ps, AF.Exp, scale=SCALE)
                        # denominator, replicated on all partitions: ones^T @ exp
                        sps = PSC.tile([128, ICH], F32, tag="sc")
                        for jb in range(SB):
                            nc.tensor.matmul(
                                sps, ones_sb, attnT[:, jb, :],
                                start=(jb == 0), stop=(jb == SB - 1))
                        recip_bc = PC1.tile([128, ICH], F32, tag="recipbc")
                        nc.vector.reciprocal(recip_bc, sps)
                        # attendedT [e_blk, i]
                        pa = PMM.tile([128, EB, ICH], F32, tag="mm")
                        for eb in range(EB):
                            for jb in range(SB):
                                nc.tensor.matmul(
                                    pa[:, eb, :], v_sb[:, jb, eb * 128:(eb + 1) * 128],
                                    attnT[:, jb, :], start=(jb == 0), stop=(jb == SB - 1))
                        attT = PC.tile([128, EB, ICH], F32R, tag="attT", bufs=1)
                        for eb in range(EB):
                            nc.vector.tensor_mul(
                                attT[:, eb, :], pa[:, eb, :], recip_bc)
                        # hT = relu(Wg1 @ attended + bg1)
                        ph = PMM.tile([128, EB, ICH], F32, tag="mm")
                        for e2 in range(EB):
                            for eb in range(EB):
                                nc.tensor.matmul(
                                    ph[:, e2, :], wg1T[:, eb, e2 * 128:(e2 + 1) * 128],
                                    attT[:, eb, :], start=(eb == 0), stop=(eb == EB - 1))
                        hT = PC.tile([128, EB, ICH], F32R, tag="hT", bufs=1)
                        for e2 in range(EB):
                            nc.scalar.activation(
                                hT[:, e2, :], ph[:, e2, :], AF.Relu,
                                bias=bg1_sb[:, e2:e2 + 1])
                        # gateT = sigmoid(Wg2 @ h + bg2); then sigmoid again
                        pg = PMM.tile([128, EB, ICH], F32, tag="mm")
                        for e2 in range(EB):
                            for eb in range(EB):
                                nc.tensor.matmul(
                                    pg[:, e2, :], wg2T[:, eb, e2 * 128:(e2 + 1) * 128],
                                    hT[:, eb, :], start=(eb == 0), stop=(eb == EB - 1))
                        g2 = PC.tile([128, EB, ICH], F32, tag="g2", bufs=1)
                        for e2 in range(EB):
                            nc.scalar.activation(
                                g2[:, e2, :], pg[:, e2, :], AF.Tanh,
                                bias=bg2h_sb[:, e2:e2 + 1], scale=0.5)
                        nc.vector.tensor_scalar(
                            g2, g2, 0.5, 0.5, mybir.AluOpType.mult,
                            mybir.AluOpType.add)
                        g3 = PC.tile([128, EB, ICH], F32, tag="g3", bufs=1)
                        nc.scalar.activation(g3, g2, AF.Tanh, scale=0.5)
                        av = PC.tile([128, EB, ICH], F32, tag="av", bufs=1)
                        for eb in range(EB):
                            nc.vector.tensor_scalar_mul(
                                av[:, eb, :], attT[:, eb, :], tsh_sb[:, eb:eb + 1])
                        gated = PC.tile([128, EB, ICH], F32R, tag="gated", bufs=1)
                        nc.vector.tensor_mul(gated, g3, av)
                        nc.vector.tensor_add(gated, gated, av)

                        # transpose back to [s, e] and store
                        for ib in range(nib):
                            po = PMM.tile([128, D], F32R, tag="mm")
                            for eb in range(EB):
                                nc.tensor.transpose(
                                    po[:, eb * 128:(eb + 1) * 128],
                                    gated[:, eb, ib * 128:(ib + 1) * 128], ident_sb)
                            osb = PC.tile([128, D], F32, tag="osb", bufs=1)
                            nc.vector.tensor_copy(osb, po)
                            r0 = (ic * nib + ib) * 128
                            nc.sync.dma_start(out=out[r0:r0 + 128, :], in_=osb)

                persist2.__exit__(None, None, None)

    nc.compile()
    return nc


def kernel(**inputs):
    if "nc" not in _CACHE:
        _CACHE["nc"] = _build()
    nc = _CACHE["nc"]
    inputs = dict(inputs)
    q = np.ascontiguousarray(inputs["query"], dtype=np.float32)
    k = np.ascontiguousarray(inputs["key"], dtype=np.float32)
    vv = np.ascontiguousarray(inputs["value"], dtype=np.float32)
    shared = {
        "Wq": np.ascontiguousarray(inputs["Wq"], np.float32),
        "Wk": np.ascontiguousarray(inputs["Wk"], np.float32),
        "Wv": np.ascontiguousarray(inputs["Wv"], np.float32),
        "Wg1": np.ascontiguousarray(inputs["Wg1"], np.float32),
        "Wg2": np.ascontiguousarray(inputs["Wg2"], np.float32),
        "bq": np.ascontiguousarray(inputs["bq"], np.float32),
        "bk": np.ascontiguousarray(inputs["bk"], np.float32),
        "bv": np.ascontiguousarray(inputs["bv"], np.float32),
        "bg1": np.ascontiguousarray(inputs["bg1"], np.float32),
        "bg2": np.ascontiguousarray(inputs["bg2"], np.float32),
        "ts": np.ascontiguousarray(inputs["text_scale"], np.float32),
        "ident": np.eye(128, dtype=np.float32),
        "ones": np.ones((128, 128), dtype=np.float32),
    }
    in_maps = [
        dict(shared, query=q[b], key=k[b], value=vv[b]) for b in range(B)
    ]
    trace = bool(inputs.get("_trace"))
    r = run_bass_kernel_spmd(nc, in_maps, list(range(B)), trace=trace)
    if trace:
        print("HW exec time:", r.exec_time_ns, "ns")
        _CACHE["last_result"] = r
    return np.stack([r.results[b]["out"] for b in range(B)], axis=0)


if __name__ == "__main__":
    rng = np.random.default_rng(0)
    pass



# revision 3
# speedup vs baseline: 1.3209x; 1.3209x over previous
"""Trainium2 Bass kernel: batched single-head attention + gate MLP.

Per-core (data-parallel over batch, 1 batch row per core):
  q = query @ Wq.T + bq ; k,v likewise
  scores = q @ k.T / sqrt(768); attn = softmax(scores)
  attended = attn @ v
  h = relu(attended @ Wg1.T + bg1); gate = sigmoid(h @ Wg2.T + bg2)
  out = sigmoid(gate) * attended * text_scale

Weights arrive pre-transposed from the host ([d, e] layout) so only the
three activation inputs are transposed on the PE. q is projected on
demand into a 3-slot SBUF ring inside the attention loop (no qT in
DRAM). v and the exp'd scores are stored bf16 (error well inside the
2e-2 budget); the v bias is folded into bg1 on the host plus a fused
(att+bv)*ts op on the Pool engine, legal because softmax rows sum to 1.
The attention loop is software-pipelined: the previous iteration's
gate tail and output transposes are emitted between the current
iteration's denominator and attended matmuls so the PE never idles.
"""
import numpy as np
import ml_dtypes

import concourse.bass as bass
import concourse.mybir as mybir
import concourse.tile as tile
from concourse import bacc
from concourse.bass_utils import run_bass_kernel_spmd

F32 = mybir.dt.float32
F32R = mybir.dt.float32r
BF16 = mybir.dt.bfloat16
AF = mybir.ActivationFunctionType
ALU = mybir.AluOpType

B, S, D = 8, 2048, 768
EB = D // 128            # 6 feature blocks
SB = S // 128            # 16 seq blocks
CH = 256                 # seq chunk = attention i-chunk
NCH = S // CH            # 8
SCALE = 1.0 / float(np.sqrt(D))

_CACHE = {}


def _build():
    nc = bacc.Bacc(None)

    query = nc.dram_tensor("query", [S, D], F32R, kind="ExternalInput")
    key = nc.dram_tensor("key", [S, D], F32R, kind="ExternalInput")
    value = nc.dram_tensor("value", [S, D], F32R, kind="ExternalInput")
    wqT = nc.dram_tensor("wqT", [D, D], F32R, kind="ExternalInput")
    wkT = nc.dram_tensor("wkT", [D, D], F32R, kind="ExternalInput")
    wvT = nc.dram_tensor("wvT", [D, D], F32R, kind="ExternalInput")
    wg1T = nc.dram_tensor("wg1T", [D, D], BF16, kind="ExternalInput")
    wg2T = nc.dram_tensor("wg2T", [D, D], BF16, kind="ExternalInput")
    bq = nc.dram_tensor("bq", [D], F32, kind="ExternalInput")
    bk = nc.dram_tensor("bk", [D], F32, kind="ExternalInput")
    bv = nc.dram_tensor("bv", [D], F32, kind="ExternalInput")
    bg1a = nc.dram_tensor("bg1a", [D], F32, kind="ExternalInput")
    bg2 = nc.dram_tensor("bg2", [D], F32, kind="ExternalInput")
    ts = nc.dram_tensor("ts", [1, D], F32, kind="ExternalInput")
    ident = nc.dram_tensor("ident", [128, 128], F32R, kind="ExternalInput")
    ones = nc.dram_tensor("ones", [128, 128], BF16, kind="ExternalInput")
    out = nc.dram_tensor("out", [S, D], F32, kind="ExternalOutput")

    with tile.TileContext(nc) as tc:
        with tc.tile_pool(name="persist", bufs=1) as P, \
             tc.tile_pool(name="psc", bufs=3, space="PSUM") as PSC, \
             tc.tile_pool(name="pmm", bufs=2, space="PSUM") as PMM, \
             tc.tile_pool(name="pdn", bufs=1, space="PSUM") as PDN, \
             tc.tile_pool(name="ppg", bufs=2, space="PSUM") as PPG, \
             tc.tile_pool(name="abq", bufs=1) as ABQ:

            ident_sb = P.tile([128, 128], F32R, tag="ident")
            nc.sync.dma_start(out=ident_sb, in_=ident[:, :])
            ones_sb = P.tile([128, 128], BF16, tag="ones")
            nc.sync.dma_start(out=ones_sb, in_=ones[:, :])

            def vec_sb(name, src):                       # [D] -> [128, EB]
                t = P.tile([128, EB], F32, tag=name, name=name)
                nc.sync.dma_start(out=t, in_=src.rearrange("(b p) -> p b", p=128))
                return t

            bq_sb = vec_sb("bq", bq[:])
            bk_sb = vec_sb("bk", bk[:])
            bv_sb = vec_sb("bv", bv[:])
            bg1_sb = vec_sb("bg1", bg1a[:])
            bg2_sb = vec_sb("bg2", bg2[:])
            ts_sb = vec_sb("ts", ts[0, :])

            kT = P.tile([128, EB, S], F32R, tag="kT")        # k^T [e, s]
            v_sb = P.tile([128, SB, D], BF16, tag="v")       # v [j, e]
            wg1_sb = P.tile([128, EB, D], BF16, tag="wg1")
            wg2_sb = P.tile([128, EB, D], BF16, tag="wg2")

            wq_sb = ABQ.tile([128, EB, D], F32R, tag="wq")

            def load_w(dst, wdram):
                nc.sync.dma_start(
                    out=dst, in_=wdram.rearrange("(db p) e -> p db e", p=128))

            # ---- staged input pipeline (key 0..7, value 0..7, query 0..7)
            order = [(key, c) for c in range(NCH)] + \
                    [(value, c) for c in range(NCH)] + \
                    [(query, c) for c in range(NCH)]
            xsts = {}

            def stage_idx(i):
                src, c = order[i]
                xst = ABQ.tile([128, 2, D], F32R, tag="xst", bufs=2)
                nc.sync.dma_start(
                    out=xst,
                    in_=src[c * CH:(c + 1) * CH, :].rearrange(
                        "(sb p) d -> p sb d", p=128))
                xsts[i] = xst

            def transpose_chunk(i):
                """PE-transpose staged chunk i -> xT [d-part, db, s]."""
                xst = xsts.pop(i)
                if i + 2 < len(order):
                    stage_idx(i + 2)
                xT = ABQ.tile([128, EB, CH], F32R, tag="xT", bufs=2)
                engs = (nc.vector, nc.gpsimd, nc.vector, nc.gpsimd)
                n = 0
                for sb in range(2):
                    for db0 in (0, 3):
                        tp = PSC.tile([128, 3, 128], F32R, tag="sc")
                        for k3 in range(3):
                            nc.tensor.transpose(
                                tp[:, k3, :],
                                xst[:, sb, (db0 + k3) * 128:(db0 + k3 + 1) * 128],
                                ident_sb)
                        engs[n].tensor_copy(
                            xT[:, db0:db0 + 3, sb * 128:(sb + 1) * 128], tp)
                        n += 1
                return xT

            def proj_T(xT, w_sb, dst, bias_sb):
                """Transposed projection: dst[:, eb, :] = (W x^T + b)[e-blk, i]."""
                for eb in range(EB):
                    mmt = PMM.tile([128, CH], F32, tag="mm")
                    for db in range(EB):
                        nc.tensor.matmul(
                            mmt, w_sb[:, db, eb * 128:(eb + 1) * 128], xT[:, db, :],
                            start=(db == 0), stop=(db == EB - 1))
                    nc.scalar.activation(
                        dst[:, eb, :], mmt, AF.Identity, bias=bias_sb[:, eb:eb + 1])

            def proj_v(xT, w_sb, c):
                """Natural projection: v[j, e] blocks, no bias (folded out)."""
                for jbh in range(2):
                    for h, (n0, n1) in enumerate(((0, 384), (384, 768))):
                        mmt = PMM.tile([128, 384], F32, tag="mm")
                        for db in range(EB):
                            nc.tensor.matmul(
                                mmt, xT[:, db, jbh * 128:(jbh + 1) * 128],
                                w_sb[:, db, n0:n1],
                                start=(db == 0), stop=(db == EB - 1))
                        eng = nc.vector if h == 0 else nc.scalar
                        if eng is nc.vector:
                            eng.tensor_copy(v_sb[:, c * 2 + jbh, n0:n1], mmt)
                        else:
                            eng.copy(v_sb[:, c * 2 + jbh, n0:n1], mmt)

            qbufs = [None] * NCH

            def process_q_chunk(c):
                xT = transpose_chunk(2 * NCH + c)
                qb = P.tile([128, EB, CH], F32R, tag="qbuf", bufs=3,
                            name=f"qbuf{c}")
                proj_T(xT, wq_sb, qb, bq_sb)
                qbufs[c] = qb

            # ---- phase AB: project key and value, then first two q chunks
            with tc.tile_pool(name="abkv", bufs=1) as ABKV:
                wk_sb = ABKV.tile([128, EB, D], F32R, tag="wk")
                wv_sb = ABKV.tile([128, EB, D], F32R, tag="wv")
                load_w(wk_sb, wkT)
                stage_idx(0)
                stage_idx(1)
                for c in range(NCH):
                    xT = transpose_chunk(c)
                    proj_T(xT, wk_sb, kT[:, :, c * CH:(c + 1) * CH], bk_sb)
                    if c == 3:
                        load_w(wv_sb, wvT)
                    if c == 6:
                        load_w(wq_sb, wqT)
                for c in range(NCH):
                    xT = transpose_chunk(NCH + c)
                    proj_v(xT, wv_sb, c)
                    if c == 5:
                        load_w(wg1_sb, wg1T)
                    if c == 6:
                        load_w(wg2_sb, wg2T)
                process_q_chunk(0)
                process_q_chunk(1)

            # ---- phase C: attention + gate, software-pipelined over i-chunks
            with tc.tile_pool(name="phc", bufs=1) as CP:
                attnT = CP.tile([128, SB, CH], BF16, tag="attnT")
                attT = CP.tile([128, EB * CH], F32R, tag="attT")
                hT = CP.tile([128, EB * CH], BF16, tag="hT")
                g2 = CP.tile([128, EB * CH], F32, tag="g2")
                g3 = CP.tile([128, EB * CH], F32, tag="g3")
                av = CP.tile([128, EB * CH], F32R, tag="av")
                gated = CP.tile([128, EB * CH], F32R, tag="gated")

                def emit_tail(j):
                    """gate tail + output transposes for iteration j."""
                    nc.scalar.activation(g3, g2, AF.Sigmoid)
                    nc.vector.tensor_mul(gated, g3, av)
                    gv = gated.rearrange("p (eb i) -> p eb i", eb=EB)
                    for ib in range(2):
                        osb = CP.tile([128, D], F32, tag="osb", bufs=2,
                                      name="osb")
                        for half in range(2):
                            po = PSC.tile([128, 3, 128], F32R, tag="sc")
                            for k3 in range(3):
                                eb = half * 3 + k3
                                nc.tensor.transpose(
                                    po[:, k3, :],
                                    gv[:, eb, ib * 128:(ib + 1) * 128], ident_sb)
                            nc.vector.tensor_copy(
                                osb[:, half * 384:(half + 1) * 384], po)
                        r0 = (j * 2 + ib) * 128
                        nc.sync.dma_start(out=out[r0:r0 + 128, :], in_=osb)

                for ic in range(NCH):
                    qb = qbufs[ic]
                    # scores^T + exp, per j-block
                    for jb in range(SB):
                        ps = PSC.tile([128, CH], F32, tag="sc")
                        for eb in range(EB):
                            nc.tensor.matmul(
                                ps, kT[:, eb, jb * 128:(jb + 1) * 128],
                                qb[:, eb, :],
                                start=(eb == 0), stop=(eb == EB - 1))
                        nc.scalar.activation(
                            attnT[:, jb, :], ps, AF.Exp, scale=SCALE)
                    # next q chunk rides in the exp shadow
                    if ic + 2 < NCH:
                        process_q_chunk(ic + 2)
                    # softmax denominator (ones^T @ exp) + reciprocal
                    dn = PDN.tile([128, CH], F32, tag="dn")
                    for jb in range(SB):
                        nc.tensor.matmul(
                            dn, ones_sb, attnT[:, jb, :],
                            start=(jb == 0), stop=(jb == SB - 1))
                    recip = CP.tile([128, CH], F32, tag="recip", bufs=2,
                                    name="recip")
                    nc.vector.reciprocal(recip, dn)
                    # previous iteration's tail fills the PE gap here
                    if ic > 0:
                        emit_tail(ic - 1)
                    # attended^T; normalize on DVE; (att+bv)*ts on Pool
                    for eb in range(EB):
                        pa = PMM.tile([128, CH], F32, tag="mm")
                        for jb in range(SB):
                            nc.tensor.matmul(
                                pa, v_sb[:, jb, eb * 128:(eb + 1) * 128],
                                attnT[:, jb, :],
                                start=(jb == 0), stop=(jb == SB - 1))
                        sl = slice(eb * CH, (eb + 1) * CH)
                        nc.vector.tensor_mul(attT[:, sl], pa, recip)
                        nc.gpsimd.tensor_scalar(
                            av[:, sl], attT[:, sl], bv_sb[:, eb:eb + 1],
                            ts_sb[:, eb:eb + 1], ALU.add, ALU.mult)
                    # gate1: h = relu(Wg1 att + bg1')
                    for e2 in range(EB):
                        ph = PPG.tile([128, CH], F32, tag="pg")
                        for eb in range(EB):
                            nc.tensor.matmul(
                                ph, wg1_sb[:, eb, e2 * 128:(e2 + 1) * 128],
                                attT[:, eb * CH:(eb + 1) * CH],
                                start=(eb == 0), stop=(eb == EB - 1))
                        nc.scalar.activation(
                            hT[:, e2 * CH:(e2 + 1) * CH], ph, AF.Relu,
                            bias=bg1_sb[:, e2:e2 + 1])
                    # gate2: sigmoid(Wg2 h + bg2)
                    for e2 in range(EB):
                        pg = PPG.tile([128, CH], F32, tag="pg")
                        for eb in range(EB):
                            nc.tensor.matmul(
                                pg, wg2_sb[:, eb, e2 * 128:(e2 + 1) * 128],
                                hT[:, eb * CH:(eb + 1) * CH],
                                start=(eb == 0), stop=(eb == EB - 1))
                        nc.scalar.activation(
                            g2[:, e2 * CH:(e2 + 1) * CH], pg, AF.Sigmoid,
                            bias=bg2_sb[:, e2:e2 + 1])
                emit_tail(NCH - 1)

    nc.compile()
    return nc


def kernel(**inputs):
    if "nc" not in _CACHE:
        _CACHE["nc"] = _build()
    nc = _CACHE["nc"]
    q = np.ascontiguousarray(inputs["query"], dtype=np.float32)
    k = np.ascontiguousarray(inputs["key"], dtype=np.float32)
    vv = np.ascontiguousarray(inputs["value"], dtype=np.float32)
    Wg1 = np.asarray(inputs["Wg1"], np.float32)
    bv_np = np.asarray(inputs["bv"], np.float32)
    bg1a = np.asarray(inputs["bg1"], np.float32) + Wg1 @ bv_np
    shared = {
        "wqT": np.ascontiguousarray(np.asarray(inputs["Wq"], np.float32).T),
        "wkT": np.ascontiguousarray(np.asarray(inputs["Wk"], np.float32).T),
        "wvT": np.ascontiguousarray(np.asarray(inputs["Wv"], np.float32).T),
        "wg1T": np.ascontiguousarray(
            Wg1.T.astype(ml_dtypes.bfloat16)),
        "wg2T": np.ascontiguousarray(
            np.asarray(inputs["Wg2"], np.float32).T.astype(ml_dtypes.bfloat16)),
        "bq": np.ascontiguousarray(inputs["bq"], np.float32),
        "bk": np.ascontiguousarray(inputs["bk"], np.float32),
        "bv": np.ascontiguousarray(bv_np),
        "bg1a": np.ascontiguousarray(bg1a),
        "bg2": np.ascontiguousarray(inputs["bg2"], np.float32),
        "ts": np.ascontiguousarray(inputs["text_scale"], np.float32),
        "ident": np.eye(128, dtype=np.float32),
        "ones": np.ones((128, 128), dtype=ml_dtypes.bfloat16),
    }
    in_maps = [
        dict(shared, query=q[b], key=k[b], value=vv[b]) for b in range(B)
    ]
    trace = bool(inputs.get("_trace"))
    r = run_bass_kernel_spmd(nc, in_maps, list(range(B)), trace=trace)
    if trace:
        print("HW exec time:", r.exec_time_ns, "ns")
        _CACHE["last_result"] = r
    return np.stack([r.results[b]["out"] for b in range(B)], axis=0)


if __name__ == "__main__":
    pass


# revision 5
# speedup vs baseline: 1.3269x; 1.0045x over previous
"""Trainium2 Bass kernel: batched single-head attention + gate MLP.

Per-core (data-parallel over batch, 1 batch row per core):
  q = query @ Wq.T + bq ; k,v likewise
  scores = q @ k.T / sqrt(768); attn = softmax(scores)
  attended = attn @ v
  h = relu(attended @ Wg1.T + bg1); gate = sigmoid(h @ Wg2.T + bg2)
  out = sigmoid(gate) * attended * text_scale

Weights arrive pre-transposed from the host ([d, e] layout) so only the
three activation inputs are transposed on the PE. q is projected on
demand into a 3-slot SBUF ring inside the attention loop (no qT in
DRAM). v and the exp'd scores are stored bf16 (error well inside the
2e-2 budget); the v bias is folded into bg1 on the host plus a fused
(att+bv)*ts op on the Pool engine, legal because softmax rows sum to 1.
The attention loop is software-pipelined: the previous iteration's
gate tail and output transposes are emitted between the current
iteration's denominator and attended matmuls so the PE never idles.
"""
import numpy as np
import ml_dtypes

import concourse.bass as bass
import concourse.mybir as mybir
import concourse.tile as tile
from concourse import bacc
from concourse.bass_utils import run_bass_kernel_spmd

F32 = mybir.dt.float32
F32R = mybir.dt.float32r
BF16 = mybir.dt.bfloat16
AF = mybir.ActivationFunctionType
ALU = mybir.AluOpType

B, S, D = 8, 2048, 768
EB = D // 128            # 6 feature blocks
SB = S // 128            # 16 seq blocks
CH = 256                 # seq chunk = attention i-chunk
NCH = S // CH            # 8
SCALE = 1.0 / float(np.sqrt(D))

_CACHE = {}


def _build():
    nc = bacc.Bacc(None)

    query = nc.dram_tensor("query", [S, D], F32R, kind="ExternalInput")
    key = nc.dram_tensor("key", [S, D], F32R, kind="ExternalInput")
    value = nc.dram_tensor("value", [S, D], F32R, kind="ExternalInput")
    wqT = nc.dram_tensor("wqT", [D, D], F32R, kind="ExternalInput")
    wkT = nc.dram_tensor("wkT", [D, D], F32R, kind="ExternalInput")
    wvT = nc.dram_tensor("wvT", [D, D], F32R, kind="ExternalInput")
    wg1T = nc.dram_tensor("wg1T", [D, D], BF16, kind="ExternalInput")
    wg2T = nc.dram_tensor("wg2T", [D, D], BF16, kind="ExternalInput")
    bq = nc.dram_tensor("bq", [D], F32, kind="ExternalInput")
    bk = nc.dram_tensor("bk", [D], F32, kind="ExternalInput")
    bv = nc.dram_tensor("bv", [D], F32, kind="ExternalInput")
    bg1a = nc.dram_tensor("bg1a", [D], F32, kind="ExternalInput")
    bg2 = nc.dram_tensor("bg2", [D], F32, kind="ExternalInput")
    ts = nc.dram_tensor("ts", [1, D], F32, kind="ExternalInput")
    ident = nc.dram_tensor("ident", [128, 128], F32R, kind="ExternalInput")
    ones = nc.dram_tensor("ones", [128, 128], BF16, kind="ExternalInput")
    out = nc.dram_tensor("out", [S, D], F32, kind="ExternalOutput")

    with tile.TileContext(nc) as tc:
        with tc.tile_pool(name="persist", bufs=1) as P, \
             tc.tile_pool(name="psc", bufs=3, space="PSUM") as PSC, \
             tc.tile_pool(name="pmm", bufs=2, space="PSUM") as PMM, \
             tc.tile_pool(name="pdn", bufs=1, space="PSUM") as PDN, \
             tc.tile_pool(name="ppg", bufs=2, space="PSUM") as PPG, \
             tc.tile_pool(name="abq", bufs=1) as ABQ:

            ident_sb = P.tile([128, 128], F32R, tag="ident")
            nc.sync.dma_start(out=ident_sb, in_=ident[:, :])
            ones_sb = P.tile([128, 128], BF16, tag="ones")
            nc.sync.dma_start(out=ones_sb, in_=ones[:, :])

            def vec_sb(name, src):                       # [D] -> [128, EB]
                t = P.tile([128, EB], F32, tag=name, name=name)
                nc.sync.dma_start(out=t, in_=src.rearrange("(b p) -> p b", p=128))
                return t

            bq_sb = vec_sb("bq", bq[:])
            bk_sb = vec_sb("bk", bk[:])
            bv_sb = vec_sb("bv", bv[:])
            bg1_sb = vec_sb("bg1", bg1a[:])
            bg2_sb = vec_sb("bg2", bg2[:])
            ts_sb = vec_sb("ts", ts[0, :])

            kT = P.tile([128, EB, S], F32R, tag="kT")        # k^T [e, s]
            v_sb = P.tile([128, SB, D], BF16, tag="v")       # v [j, e]
            wg1_sb = P.tile([128, EB, D], BF16, tag="wg1")
            wg2_sb = P.tile([128, EB, D], BF16, tag="wg2")

            wq_sb = ABQ.tile([128, EB, D], F32R, tag="wq")

            def load_w(dst, wdram):
                nc.sync.dma_start(
                    out=dst, in_=wdram.rearrange("(db p) e -> p db e", p=128))

            # ---- staged input pipeline (key 0..7, value 0..7, query 0..7)
            order = [(key, c) for c in range(NCH)] + \
                    [(value, c) for c in range(NCH)] + \
                    [(query, c) for c in range(NCH)]
            xsts = {}

            def stage_idx(i):
                src, c = order[i]
                xst = ABQ.tile([128, 2, D], F32R, tag="xst", bufs=2)
                nc.sync.dma_start(
                    out=xst,
                    in_=src[c * CH:(c + 1) * CH, :].rearrange(
                        "(sb p) d -> p sb d", p=128))
                xsts[i] = xst

            def transpose_chunk(i):
                """PE-transpose staged chunk i -> xT [d-part, db, s]."""
                xst = xsts.pop(i)
                if i + 2 < len(order):
                    stage_idx(i + 2)
                xT = ABQ.tile([128, EB, CH], F32R, tag="xT", bufs=2)
                n = 0
                for sb in range(2):
                    for db0 in (0, 3):
                        tp = PSC.tile([128, 3, 128], F32R, tag="sc")
                        for k3 in range(3):
                            nc.tensor.transpose(
                                tp[:, k3, :],
                                xst[:, sb, (db0 + k3) * 128:(db0 + k3 + 1) * 128],
                                ident_sb)
                        dst = xT[:, db0:db0 + 3, sb * 128:(sb + 1) * 128]
                        if n == 1:
                            nc.scalar.copy(dst, tp)
                        else:
                            nc.vector.tensor_copy(dst, tp)
                        n += 1
                return xT

            def proj_T(xT, w_sb, dst, bias_sb):
                """Transposed projection: dst[:, eb, :] = (W x^T + b)[e-blk, i]."""
                for eb in range(EB):
                    mmt = PMM.tile([128, CH], F32, tag="mm")
                    for db in range(EB):
                        nc.tensor.matmul(
                            mmt, w_sb[:, db, eb * 128:(eb + 1) * 128], xT[:, db, :],
                            start=(db == 0), stop=(db == EB - 1))
                    nc.scalar.activation(
                        dst[:, eb, :], mmt, AF.Identity, bias=bias_sb[:, eb:eb + 1])

            def proj_v(xT, w_sb, c):
                """Natural projection: v[j, e] blocks, no bias (folded out)."""
                for jbh in range(2):
                    for h, (n0, n1) in enumerate(((0, 384), (384, 768))):
                        mmt = PMM.tile([128, 384], F32, tag="mm")
                        for db in range(EB):
                            nc.tensor.matmul(
                                mmt, xT[:, db, jbh * 128:(jbh + 1) * 128],
                                w_sb[:, db, n0:n1],
                                start=(db == 0), stop=(db == EB - 1))
                        eng = nc.vector if h == 0 else nc.scalar
                        if eng is nc.vector:
                            eng.tensor_copy(v_sb[:, c * 2 + jbh, n0:n1], mmt)
                        else:
                            eng.copy(v_sb[:, c * 2 + jbh, n0:n1], mmt)

            qbufs = [None] * NCH

            def process_q_chunk(c):
                xT = transpose_chunk(2 * NCH + c)
                qb = P.tile([128, EB, CH], F32R, tag="qbuf", bufs=3,
                            name=f"qbuf{c}")
                proj_T(xT, wq_sb, qb, bq_sb)
                qbufs[c] = qb

            # ---- phase AB: project key and value, then first two q chunks
            with tc.tile_pool(name="abkv", bufs=1) as ABKV:
                wk_sb = ABKV.tile([128, EB, D], F32R, tag="wk")
                wv_sb = ABKV.tile([128, EB, D], F32R, tag="wv")
                load_w(wk_sb, wkT)
                stage_idx(0)
                stage_idx(1)
                for c in range(NCH):
                    xT = transpose_chunk(c)
                    proj_T(xT, wk_sb, kT[:, :, c * CH:(c + 1) * CH], bk_sb)
                    if c == 3:
                        load_w(wv_sb, wvT)
                    if c == 6:
                        load_w(wq_sb, wqT)
                for c in range(NCH):
                    xT = transpose_chunk(NCH + c)
                    proj_v(xT, wv_sb, c)
                    if c == 5:
                        load_w(wg1_sb, wg1T)
                    if c == 6:
                        load_w(wg2_sb, wg2T)
                process_q_chunk(0)
                process_q_chunk(1)

            # ---- phase C: attention + gate, software-pipelined over i-chunks
            with tc.tile_pool(name="phc", bufs=1) as CP:
                attnT = CP.tile([128, SB, CH], BF16, tag="attnT")
                attT = CP.tile([128, EB * CH], BF16, tag="attT")
                hT = CP.tile([128, EB * CH], BF16, tag="hT")
                g2 = CP.tile([128, EB * CH], F32, tag="g2")
                g3 = CP.tile([128, EB * CH], F32, tag="g3")
                av = CP.tile([128, EB * CH], F32R, tag="av")
                gated = CP.tile([128, EB * CH], F32R, tag="gated")

                def emit_tail(j):
                    """gate tail + output transposes for iteration j."""
                    nc.scalar.activation(g3, g2, AF.Sigmoid)
                    nc.vector.tensor_mul(gated, g3, av)
                    gv = gated.rearrange("p (eb i) -> p eb i", eb=EB)
                    for ib in range(2):
                        osb = CP.tile([128, D], F32, tag="osb", bufs=2,
                                      name="osb")
                        for half in range(2):
                            po = PSC.tile([128, 3, 128], F32R, tag="sc")
                            for k3 in range(3):
                                eb = half * 3 + k3
                                nc.tensor.transpose(
                                    po[:, k3, :],
                                    gv[:, eb, ib * 128:(ib + 1) * 128], ident_sb)
                            nc.vector.tensor_copy(
                                osb[:, half * 384:(half + 1) * 384], po)
                        r0 = (j * 2 + ib) * 128
                        nc.sync.dma_start(out=out[r0:r0 + 128, :], in_=osb)

                for ic in range(NCH):
                    qb = qbufs[ic]
                    # scores^T + exp, per j-block
                    for jb in range(SB):
                        ps = PSC.tile([128, CH], F32, tag="sc")
                        for eb in range(EB):
                            nc.tensor.matmul(
                                ps, kT[:, eb, jb * 128:(jb + 1) * 128],
                                qb[:, eb, :],
                                start=(eb == 0), stop=(eb == EB - 1))
                        nc.scalar.activation(
                            attnT[:, jb, :], ps, AF.Exp, scale=SCALE)
                    # next q chunk rides in the exp shadow
                    if ic + 2 < NCH:
                        process_q_chunk(ic + 2)
                    # softmax denominator (ones^T @ exp) + reciprocal
                    dn = PDN.tile([128, CH], F32, tag="dn")
                    for jb in range(SB):
                        nc.tensor.matmul(
                            dn, ones_sb, attnT[:, jb, :],
                            start=(jb == 0), stop=(jb == SB - 1))
                    recip = CP.tile([128, CH], F32, tag="recip", bufs=2,
                                    name="recip")
                    nc.vector.reciprocal(recip, dn)
                    # previous iteration's tail fills the PE gap here
                    if ic > 0:
                        emit_tail(ic - 1)
                    # attended^T; normalize on DVE; (att+bv)*ts on Pool
                    for eb in range(EB):
                        pa = PMM.tile([128, CH], F32, tag="mm")
                        for jb in range(SB):
                            nc.tensor.matmul(
                                pa, v_sb[:, jb, eb * 128:(eb + 1) * 128],
                                attnT[:, jb, :],
                                start=(jb == 0), stop=(jb == SB - 1))
                        sl = slice(eb * CH, (eb + 1) * CH)
                        nc.vector.tensor_mul(attT[:, sl], pa, recip)
                        nc.gpsimd.tensor_scalar(
                            av[:, sl], attT[:, sl], bv_sb[:, eb:eb + 1],
                            ts_sb[:, eb:eb + 1], ALU.add, ALU.mult)
                    # gate1: h = relu(Wg1 att + bg1')
                    for e2 in range(EB):
                        ph = PPG.tile([128, CH], F32, tag="pg")
                        for eb in range(EB):
                            nc.tensor.matmul(
                                ph, wg1_sb[:, eb, e2 * 128:(e2 + 1) * 128],
                                attT[:, eb * CH:(eb + 1) * CH],
                                start=(eb == 0), stop=(eb == EB - 1))
                        nc.scalar.activation(
                            hT[:, e2 * CH:(e2 + 1) * CH], ph, AF.Relu,
                            bias=bg1_sb[:, e2:e2 + 1])
                    # gate2: sigmoid(Wg2 h + bg2)
                    for e2 in range(EB):
                        pg = PPG.tile([128, CH], F32, tag="pg")
                        for eb in range(EB):
                            nc.tensor.matmul(
                                pg, wg2_sb[:, eb, e2 * 128:(e2 + 1) * 128],
                                hT[:, eb * CH:(eb + 1) * CH],
                                start=(eb == 0), stop=(eb == EB - 1))
                        nc.scalar.activation(
                            g2[:, e2 * CH:(e2 + 1) * CH], pg, AF.Sigmoid,
                            bias=bg2_sb[:, e2:e2 + 1])
                emit_tail(NCH - 1)

    nc.compile()
    return nc


def kernel(**inputs):
    if "nc" not in _CACHE:
        _CACHE["nc"] = _build()
    nc = _CACHE["nc"]
    q = np.ascontiguousarray(inputs["query"], dtype=np.float32)
    k = np.ascontiguousarray(inputs["key"], dtype=np.float32)
    vv = np.ascontiguousarray(inputs["value"], dtype=np.float32)
    Wg1 = np.asarray(inputs["Wg1"], np.float32)
    bv_np = np.asarray(inputs["bv"], np.float32)
    bg1a = np.asarray(inputs["bg1"], np.float32) + Wg1 @ bv_np
    shared = {
        "wqT": np.ascontiguousarray(np.asarray(inputs["Wq"], np.float32).T),
        "wkT": np.ascontiguousarray(np.asarray(inputs["Wk"], np.float32).T),
        "wvT": np.ascontiguousarray(np.asarray(inputs["Wv"], np.float32).T),
        "wg1T": np.ascontiguousarray(
            Wg1.T.astype(ml_dtypes.bfloat16)),
        "wg2T": np.ascontiguousarray(
            np.asarray(inputs["Wg2"], np.float32).T.astype(ml_dtypes.bfloat16)),
        "bq": np.ascontiguousarray(inputs["bq"], np.float32),
        "bk": np.ascontiguousarray(inputs["bk"], np.float32),
        "bv": np.ascontiguousarray(bv_np),
        "bg1a": np.ascontiguousarray(bg1a),
        "bg2": np.ascontiguousarray(inputs["bg2"], np.float32),
        "ts": np.ascontiguousarray(inputs["text_scale"], np.float32),
        "ident": np.eye(128, dtype=np.float32),
        "ones": np.ones((128, 128), dtype=ml_dtypes.bfloat16),
    }
    in_maps = [
        dict(shared, query=q[b], key=k[b], value=vv[b]) for b in range(B)
    ]
    trace = bool(inputs.get("_trace"))
    r = run_bass_kernel_spmd(nc, in_maps, list(range(B)), trace=trace)
    if trace:
        print("HW exec time:", r.exec_time_ns, "ns")
        _CACHE["last_result"] = r
    return np.stack([r.results[b]["out"] for b in range(B)], axis=0)


if __name__ == "__main__":
    pass


# revision 14
# speedup vs baseline: 1.5053x; 1.1345x over previous
"""Trainium2 Bass kernel: batched single-head attention + gate MLP.

Per-core (data-parallel over batch, 1 batch row per core):
  q = query @ Wq.T + bq ; k,v likewise
  scores = q @ k.T / sqrt(768); attn = softmax(scores)
  attended = attn @ v
  h = relu(attended @ Wg1.T + bg1); gate = sigmoid(h @ Wg2.T + bg2)
  out = sigmoid(gate) * attended * text_scale

Weights arrive pre-transposed from the host ([d, e] layout) so only the
three activation inputs are transposed on the PE. q is projected on
demand into a 3-slot SBUF ring inside the attention loop (no qT in
DRAM). v and the exp'd scores are stored bf16 (error well inside the
2e-2 budget); the v bias is folded into bg1 on the host plus a fused
(att+bv)*ts op on the Pool engine, legal because softmax rows sum to 1.
The attention loop is software-pipelined: the previous iteration's
gate tail and output transposes are emitted between the current
iteration's denominator and attended matmuls so the PE never idles.
"""
import numpy as np
import ml_dtypes

import concourse.bass as bass
import concourse.mybir as mybir
import concourse.tile as tile
from concourse import bacc
from concourse.bass_utils import run_bass_kernel_spmd

F32 = mybir.dt.float32
F32R = mybir.dt.float32r
BF16 = mybir.dt.bfloat16
AF = mybir.ActivationFunctionType
ALU = mybir.AluOpType

B, S, D = 8, 2048, 768
EB = D // 128            # 6 feature blocks
SB = S // 128            # 16 seq blocks
CH = 256                 # seq chunk = attention i-chunk
NCH = S // CH            # 8
SCALE = 1.0 / float(np.sqrt(D))

_CACHE = {}


def _build():
    nc = bacc.Bacc(None)

    query = nc.dram_tensor("query", [S, D], F32R, kind="ExternalInput")
    key = nc.dram_tensor("key", [S, D], F32R, kind="ExternalInput")
    value = nc.dram_tensor("value", [S, D], F32R, kind="ExternalInput")
    wqT = nc.dram_tensor("wqT", [D, D], F32R, kind="ExternalInput")
    wkT = nc.dram_tensor("wkT", [D, D], F32R, kind="ExternalInput")
    wvT = nc.dram_tensor("wvT", [D, D], F32R, kind="ExternalInput")
    wg1T = nc.dram_tensor("wg1T", [D, D], BF16, kind="ExternalInput")
    wg2T = nc.dram_tensor("wg2T", [D, D], BF16, kind="ExternalInput")
    bq = nc.dram_tensor("bq", [D], F32, kind="ExternalInput")
    bk = nc.dram_tensor("bk", [D], F32, kind="ExternalInput")
    bv = nc.dram_tensor("bv", [D], F32, kind="ExternalInput")
    bg1a = nc.dram_tensor("bg1a", [D], F32, kind="ExternalInput")
    bg2 = nc.dram_tensor("bg2", [D], F32, kind="ExternalInput")
    ts = nc.dram_tensor("ts", [1, D], F32, kind="ExternalInput")
    ident = nc.dram_tensor("ident", [128, 128], F32R, kind="ExternalInput")
    ones = nc.dram_tensor("ones", [128, 128], BF16, kind="ExternalInput")
    out = nc.dram_tensor("out", [S, D], F32, kind="ExternalOutput")

    with tile.TileContext(nc) as tc:
        with tc.tile_pool(name="persist", bufs=1) as P, \
             tc.tile_pool(name="psc", bufs=3, space="PSUM") as PSC, \
             tc.tile_pool(name="pmm", bufs=2, space="PSUM") as PMM, \
             tc.tile_pool(name="pdn", bufs=1, space="PSUM") as PDN, \
             tc.tile_pool(name="ppg", bufs=2, space="PSUM") as PPG, \
             tc.tile_pool(name="abq", bufs=1) as ABQ:

            ident_sb = P.tile([128, 128], F32R, tag="ident")
            nc.sync.dma_start(out=ident_sb, in_=ident[:, :])
            c25_sb = P.tile([128, 1], F32, tag="c25")
            nc.vector.memset(c25_sb, 0.25)

            def vec_sb(name, src):                       # [D] -> [128, EB]
                t = P.tile([128, EB], F32, tag=name, name=name)
                nc.sync.dma_start(out=t, in_=src.rearrange("(b p) -> p b", p=128))
                return t

            kT = P.tile([128, EB, S], F32R, tag="kT")        # k^T [e, s]
            v_sb = P.tile([128, SB, D], BF16, tag="v")       # v [j, e]
            wg1_sb = P.tile([128, EB, D], BF16, tag="wg1")
            wg2_sb = P.tile([128, EB, D], BF16, tag="wg2")

            wq_sb = ABQ.tile([128, EB, D], F32R, tag="wq")

            def load_w(dst, wdram):
                nc.sync.dma_start(
                    out=dst, in_=wdram.rearrange("(db p) e -> p db e", p=128))

            # ---- staged input pipeline (key 0..7, value 0..7, query 0..7)
            order = [(key, c) for c in range(NCH)] + \
                    [(value, c) for c in range(NCH)] + \
                    [(query, c) for c in range(NCH)]
            xsts = {}

            def stage_idx(i):
                src, c = order[i]
                xst = ABQ.tile([128, 2, D], F32R, tag="xst", bufs=2)
                nc.sync.dma_start(
                    out=xst,
                    in_=src[c * CH:(c + 1) * CH, :].rearrange(
                        "(sb p) d -> p sb d", p=128))
                xsts[i] = xst

            def transpose_chunk(i):
                """PE-transpose staged chunk i -> xT [d-part, db, s]."""
                xst = xsts.pop(i)
                if i + 2 < len(order):
                    stage_idx(i + 2)
                xT = ABQ.tile([128, EB, CH], F32R, tag="xT", bufs=2)
                n = 0
                for sb in range(2):
                    for db0 in (0, 3):
                        tp = PSC.tile([128, 3, 128], F32R, tag="sc")
                        for k3 in range(3):
                            nc.tensor.transpose(
                                tp[:, k3, :],
                                xst[:, sb, (db0 + k3) * 128:(db0 + k3 + 1) * 128],
                                ident_sb)
                        dst = xT[:, db0:db0 + 3, sb * 128:(sb + 1) * 128]
                        if n == 1:
                            nc.scalar.copy(dst, tp)
                        else:
                            nc.vector.tensor_copy(dst, tp)
                        n += 1
                return xT

            def proj_T(xT, w_sb, dst, bias_sb):
                """Transposed projection: dst[:, eb, :] = (W x^T + b)[e-blk, i]."""
                for eb in range(EB):
                    mmt = PMM.tile([128, CH], F32, tag="mm")
                    for db in range(EB):
                        nc.tensor.matmul(
                            mmt, w_sb[:, db, eb * 128:(eb + 1) * 128], xT[:, db, :],
                            start=(db == 0), stop=(db == EB - 1))
                    nc.scalar.activation(
                        dst[:, eb, :], mmt, AF.Identity, bias=bias_sb[:, eb:eb + 1])

            def proj_v(xT, w_sb, c):
                """Natural projection: v[j, e] blocks, no bias (folded out)."""
                for jbh in range(2):
                    for h, (n0, n1) in enumerate(((0, 384), (384, 768))):
                        mmt = PMM.tile([128, 384], F32, tag="mm")
                        for db in range(EB):
                            nc.tensor.matmul(
                                mmt, xT[:, db, jbh * 128:(jbh + 1) * 128],
                                w_sb[:, db, n0:n1],
                                start=(db == 0), stop=(db == EB - 1))
                        eng = nc.vector if h == 0 else nc.scalar
                        if eng is nc.vector:
                            eng.tensor_copy(v_sb[:, c * 2 + jbh, n0:n1], mmt)
                        else:
                            eng.copy(v_sb[:, c * 2 + jbh, n0:n1], mmt)

            qbufs = [None] * NCH

            def process_q_chunk(c):
                xT = transpose_chunk(2 * NCH + c)
                qb = P.tile([128, EB, CH], F32R, tag="qbuf", bufs=3,
                            name=f"qbuf{c}")
                proj_T(xT, wq_sb, qb, bq_sb)
                qbufs[c] = qb

            # ---- phase AB: project key and value, then first two q chunks
            with tc.tile_pool(name="abkv", bufs=1) as ABKV:
                wk_sb = ABKV.tile([128, EB, D], F32R, tag="wk")
                wv_sb = ABKV.tile([128, EB, D], F32R, tag="wv")
                load_w(wk_sb, wkT)
                stage_idx(0)
                stage_idx(1)
                # small constants after the critical-path loads
                ones_sb = P.tile([128, 128], BF16, tag="ones")
                nc.sync.dma_start(out=ones_sb, in_=ones[:, :])
                bq_sb = vec_sb("bq", bq[:])
                bk_sb = vec_sb("bk", bk[:])
                bv_sb = vec_sb("bv", bv[:])
                bg1_sb = vec_sb("bg1", bg1a[:])
                bg2_sb = vec_sb("bg2", bg2[:])      # host passes bg2/2
                ts_sb = vec_sb("ts", ts[0, :])      # host passes ts/2
                for c in range(NCH):
                    xT = transpose_chunk(c)
                    proj_T(xT, wk_sb, kT[:, :, c * CH:(c + 1) * CH], bk_sb)
                    if c == 3:
                        load_w(wv_sb, wvT)
                    if c == 6:
                        load_w(wq_sb, wqT)
                for c in range(NCH):
                    xT = transpose_chunk(NCH + c)
                    proj_v(xT, wv_sb, c)
                    if c == 5:
                        load_w(wg1_sb, wg1T)
                    if c == 6:
                        load_w(wg2_sb, wg2T)
                process_q_chunk(0)
                process_q_chunk(1)

            # ---- phase C: attention + gate, software-pipelined over i-chunks
            with tc.tile_pool(name="phc", bufs=1) as CP:
                attnT = CP.tile([128, SB, CH], BF16, tag="attnT")
                attTb = CP.tile([128, EB * CH], BF16, tag="attTb")   # gate path
                attTf = CP.tile([128, EB * CH], F32R, tag="attTf")   # output path
                hT = CP.tile([128, EB * CH], BF16, tag="hT")
                g2 = CP.tile([128, EB * CH], F32, tag="g2")          # tanh(gate/2)
                g3 = CP.tile([128, EB * CH], F32, tag="g3")          # tanh(s1/2)
                av = CP.tile([128, EB * CH], F32R, tag="av")         # (att+bv)*ts/2
                gated = CP.tile([128, EB * CH], F32R, tag="gated")

                def emit_tail(j):
                    """gate tail + output transposes for iteration j.

                    s1 = sigmoid(gate) = .5 + .5*g2 ; s2 = sigmoid(s1)
                    out = s2*att*ts = (1 + tanh(.25*g2 + .25)) * (att+bv)*ts/2
                    """
                    nc.scalar.activation(g3, g2, AF.Tanh, bias=c25_sb, scale=0.25)
                    nc.vector.scalar_tensor_tensor(
                        gated, g3, 1.0, av, ALU.add, ALU.mult)
                    gv = gated.rearrange("p (eb i) -> p eb i", eb=EB)
                    for ib in range(2):
                        osb = CP.tile([128, D], F32, tag="osb", bufs=2,
                                      name="osb")
                        for half in range(2):
                            po = PSC.tile([128, 3, 128], F32R, tag="sc")
                            for k3 in range(3):
                                eb = half * 3 + k3
                                nc.tensor.transpose(
                                    po[:, k3, :],
                                    gv[:, eb, ib * 128:(ib + 1) * 128], ident_sb)
                            nc.vector.tensor_copy(
                                osb[:, half * 384:(half + 1) * 384], po)
                        r0 = (j * 2 + ib) * 128
                        nc.sync.dma_start(out=out[r0:r0 + 128, :], in_=osb)

                for ic in range(NCH):
                    qb = qbufs[ic]
                    # scores^T + exp, per j-block
                    for jb in range(SB):
                        ps = PSC.tile([128, CH], F32, tag="sc")
                        for eb in range(EB):
                            nc.tensor.matmul(
                                ps, kT[:, eb, jb * 128:(jb + 1) * 128],
                                qb[:, eb, :],
                                start=(eb == 0), stop=(eb == EB - 1))
                        nc.scalar.activation(
                            attnT[:, jb, :], ps, AF.Exp, scale=SCALE)
                    # next q chunk rides in the exp shadow
                    if ic + 2 < NCH:
                        process_q_chunk(ic + 2)
                    # softmax denominator (ones^T @ exp) + reciprocal
                    dn = PDN.tile([128, CH], F32, tag="dn")
                    for jb in range(SB):
                        nc.tensor.matmul(
                            dn, ones_sb, attnT[:, jb, :],
                            start=(jb == 0), stop=(jb == SB - 1))
                    recip = CP.tile([128, CH], F32, tag="recip", bufs=2,
                                    name="recip")
                    nc.vector.reciprocal(recip, dn)
                    # previous iteration's tail fills the PE gap here
                    if ic > 0:
                        emit_tail(ic - 1)
                    # attended^T; normalize on DVE; (att+bv)*ts on Pool
                    for eb in range(EB):
                        pa = PMM.tile([128, CH], F32, tag="mm")
                        for jb in range(SB):
                            nc.tensor.matmul(
                                pa, v_sb[:, jb, eb * 128:(eb + 1) * 128],
                                attnT[:, jb, :],
                                start=(jb == 0), stop=(jb == SB - 1))
                        sl = slice(eb * CH, (eb + 1) * CH)
                        nc.vector.tensor_mul(attTb[:, sl], pa, recip)
                        nc.vector.tensor_mul(attTf[:, sl], pa, recip)
                        nc.gpsimd.tensor_scalar(
                            av[:, sl], attTf[:, sl], bv_sb[:, eb:eb + 1],
                            ts_sb[:, eb:eb + 1], ALU.add, ALU.mult)
                    # gate1: h = relu(Wg1 att + bg1')
                    for e2 in range(EB):
                        ph = PPG.tile([128, CH], F32, tag="pg")
                        for eb in range(EB):
                            nc.tensor.matmul(
                                ph, wg1_sb[:, eb, e2 * 128:(e2 + 1) * 128],
                                attTb[:, eb * CH:(eb + 1) * CH],
                                start=(eb == 0), stop=(eb == EB - 1))
                        nc.scalar.activation(
                            hT[:, e2 * CH:(e2 + 1) * CH], ph, AF.Relu,
                            bias=bg1_sb[:, e2:e2 + 1])
                    # gate2: sigmoid(Wg2 h + bg2)
                    for e2 in range(EB):
                        pg = PPG.tile([128, CH], F32, tag="pg")
                        for eb in range(EB):
                            nc.tensor.matmul(
                                pg, wg2_sb[:, eb, e2 * 128:(e2 + 1) * 128],
                                hT[:, eb * CH:(eb + 1) * CH],
                                start=(eb == 0), stop=(eb == EB - 1))
                        nc.scalar.activation(
                            g2[:, e2 * CH:(e2 + 1) * CH], pg, AF.Tanh,
                            bias=bg2_sb[:, e2:e2 + 1], scale=0.5)
                emit_tail(NCH - 1)

    nc.compile()
    return nc


def kernel(**inputs):
    if "nc" not in _CACHE:
        _CACHE["nc"] = _build()
    nc = _CACHE["nc"]
    q = np.ascontiguousarray(inputs["query"], dtype=np.float32)
    k = np.ascontiguousarray(inputs["key"], dtype=np.float32)
    vv = np.ascontiguousarray(inputs["value"], dtype=np.float32)
    Wg1 = np.asarray(inputs["Wg1"], np.float32)
    bv_np = np.asarray(inputs["bv"], np.float32)
    bg1a = np.asarray(inputs["bg1"], np.float32) + Wg1 @ bv_np
    shared = {
        "wqT": np.ascontiguousarray(np.asarray(inputs["Wq"], np.float32).T),
        "wkT": np.ascontiguousarray(np.asarray(inputs["Wk"], np.float32).T),
        "wvT": np.ascontiguousarray(np.asarray(inputs["Wv"], np.float32).T),
        "wg1T": np.ascontiguousarray(
            Wg1.T.astype(ml_dtypes.bfloat16)),
        "wg2T": np.ascontiguousarray(
            np.asarray(inputs["Wg2"], np.float32).T.astype(ml_dtypes.bfloat16)),
        "bq": np.ascontiguousarray(inputs["bq"], np.float32),
        "bk": np.ascontiguousarray(inputs["bk"], np.float32),
        "bv": np.ascontiguousarray(bv_np),
        "bg1a": np.ascontiguousarray(bg1a),
        "bg2": np.ascontiguousarray(
            np.asarray(inputs["bg2"], np.float32) * 0.5),
        "ts": np.ascontiguousarray(
            np.asarray(inputs["text_scale"], np.float32) * 0.5),
        "ident": np.eye(128, dtype=np.float32),
        "ones": np.ones((128, 128), dtype=ml_dtypes.bfloat16),
    }
    in_maps = [
        dict(shared, query=q[b], key=k[b], value=vv[b]) for b in range(B)
    ]
    trace = bool(inputs.get("_trace"))
    r = run_bass_kernel_spmd(nc, in_maps, list(range(B)), trace=trace)
    if trace:
        print("HW exec time:", r.exec_time_ns, "ns")
        _CACHE["last_result"] = r
    return np.stack([r.results[b]["out"] for b in range(B)], axis=0)


if __name__ == "__main__":
    pass


# revision 15
# speedup vs baseline: 1.5455x; 1.0267x over previous
"""Trainium2 Bass kernel: batched single-head attention + gate MLP.

Per-core (data-parallel over batch, 1 batch row per core):
  q = query @ Wq.T + bq ; k,v likewise
  scores = q @ k.T / sqrt(768); attn = softmax(scores)
  attended = attn @ v
  h = relu(attended @ Wg1.T + bg1); gate = sigmoid(h @ Wg2.T + bg2)
  out = sigmoid(gate) * attended * text_scale

Weights arrive pre-transposed from the host ([d, e] layout) so only the
three activation inputs are transposed on the PE. q is projected on
demand into a 3-slot SBUF ring inside the attention loop (no qT in
DRAM). v and the exp'd scores are stored bf16; the normalized attended
is evicted twice (bf16 for the gate matmul, f32r for the output path)
so the output is never quantized below f32r. The v bias is folded into
bg1 on the host plus a fused (att+bv)*(ts/2) op on the Pool engine,
legal because softmax rows sum to 1. Sigmoids use the tanh half-angle
identity so every activation lives in one act-function table set
(exp_and_others) — a single table load for the whole kernel.

Scheduling: one software-pipelined chunk loop (transpose chunk i+2
after projecting chunk i) keeps the PE fed through the projections;
in the attention loop the previous iteration's gate tail + output
transposes are emitted between attended and gate1, and the last
iteration runs a per-block tail to shorten the drain.
"""
import numpy as np
import ml_dtypes

import concourse.bass as bass
import concourse.mybir as mybir
import concourse.tile as tile
from concourse import bacc
from concourse.bass_utils import run_bass_kernel_spmd

F32 = mybir.dt.float32
F32R = mybir.dt.float32r
BF16 = mybir.dt.bfloat16
AF = mybir.ActivationFunctionType
ALU = mybir.AluOpType

B, S, D = 8, 2048, 768
EB = D // 128            # 6 feature blocks
SB = S // 128            # 16 seq blocks
CH = 256                 # seq chunk = attention i-chunk
NCH = S // CH            # 8
SCALE = 1.0 / float(np.sqrt(D))

_CACHE = {}


def _build():
    nc = bacc.Bacc(None)

    query = nc.dram_tensor("query", [S, D], F32R, kind="ExternalInput")
    key = nc.dram_tensor("key", [S, D], F32R, kind="ExternalInput")
    value = nc.dram_tensor("value", [S, D], F32R, kind="ExternalInput")
    wqT = nc.dram_tensor("wqT", [D, D], F32R, kind="ExternalInput")
    wkT = nc.dram_tensor("wkT", [D, D], F32R, kind="ExternalInput")
    wvT = nc.dram_tensor("wvT", [D, D], F32R, kind="ExternalInput")
    wg1T = nc.dram_tensor("wg1T", [D, D], BF16, kind="ExternalInput")
    wg2T = nc.dram_tensor("wg2T", [D, D], BF16, kind="ExternalInput")
    bq = nc.dram_tensor("bq", [D], F32, kind="ExternalInput")
    bk = nc.dram_tensor("bk", [D], F32, kind="ExternalInput")
    bv = nc.dram_tensor("bv", [D], F32, kind="ExternalInput")
    bg1a = nc.dram_tensor("bg1a", [D], F32, kind="ExternalInput")
    bg2 = nc.dram_tensor("bg2", [D], F32, kind="ExternalInput")
    ts = nc.dram_tensor("ts", [1, D], F32, kind="ExternalInput")
    ident = nc.dram_tensor("ident", [128, 128], F32R, kind="ExternalInput")
    ones = nc.dram_tensor("ones", [128, 128], BF16, kind="ExternalInput")
    out = nc.dram_tensor("out", [S, D], F32, kind="ExternalOutput")

    with tile.TileContext(nc) as tc:
        with tc.tile_pool(name="persist", bufs=1) as P, \
             tc.tile_pool(name="psc", bufs=3, space="PSUM") as PSC, \
             tc.tile_pool(name="pmm", bufs=2, space="PSUM") as PMM, \
             tc.tile_pool(name="pdn", bufs=1, space="PSUM") as PDN, \
             tc.tile_pool(name="ppg", bufs=2, space="PSUM") as PPG, \
             tc.tile_pool(name="abq", bufs=1) as ABQ:

            ident_sb = P.tile([128, 128], F32R, tag="ident")
            nc.sync.dma_start(out=ident_sb, in_=ident[:, :])
            c25_sb = P.tile([128, 1], F32, tag="c25")
            nc.vector.memset(c25_sb, 0.25)

            def vec_sb(name, src):                       # [D] -> [128, EB]
                t = P.tile([128, EB], F32, tag=name, name=name)
                nc.sync.dma_start(out=t, in_=src.rearrange("(b p) -> p b", p=128))
                return t

            kT = P.tile([128, EB, S], F32R, tag="kT")        # k^T [e, s]
            v_sb = P.tile([128, SB, D], BF16, tag="v")       # v [j, e]
            wg1_sb = P.tile([128, EB, D], BF16, tag="wg1")
            wg2_sb = P.tile([128, EB, D], BF16, tag="wg2")

            wq_sb = ABQ.tile([128, EB, D], F32R, tag="wq")

            def load_w(dst, wdram):
                nc.sync.dma_start(
                    out=dst, in_=wdram.rearrange("(db p) e -> p db e", p=128))

            # ---- staged input pipeline (key 0..7, value 8..15, query 16..23)
            order = [(key, c) for c in range(NCH)] + \
                    [(value, c) for c in range(NCH)] + \
                    [(query, c) for c in range(NCH)]
            xsts = {}
            xTs = {}

            def stage_idx(i):
                src, c = order[i]
                xst = ABQ.tile([128, 2, D], F32R, tag="xst", bufs=2)
                nc.sync.dma_start(
                    out=xst,
                    in_=src[c * CH:(c + 1) * CH, :].rearrange(
                        "(sb p) d -> p sb d", p=128))
                xsts[i] = xst

            def trans_chunk(i):
                """PE-transpose staged chunk i -> xT [d-part, db, s]."""
                xst = xsts.pop(i)
                if i + 2 < len(order):
                    stage_idx(i + 2)
                xT = ABQ.tile([128, EB, CH], F32R, tag="xT", bufs=2)
                n = 0
                for sb in range(2):
                    for db0 in (0, 3):
                        tp = PSC.tile([128, 3, 128], F32R, tag="sc")
                        for k3 in range(3):
                            nc.tensor.transpose(
                                tp[:, k3, :],
                                xst[:, sb, (db0 + k3) * 128:(db0 + k3 + 1) * 128],
                                ident_sb)
                        dst = xT[:, db0:db0 + 3, sb * 128:(sb + 1) * 128]
                        if n == 1:
                            nc.scalar.copy(dst, tp)
                        else:
                            nc.vector.tensor_copy(dst, tp)
                        n += 1
                xTs[i] = xT

            def proj_T(xT, w_sb, dst, bias_sb):
                """Transposed projection: dst[:, eb, :] = (W x^T + b)[e-blk, i]."""
                for eb in range(EB):
                    mmt = PMM.tile([128, CH], F32, tag="mm")
                    for db in range(EB):
                        nc.tensor.matmul(
                            mmt, w_sb[:, db, eb * 128:(eb + 1) * 128], xT[:, db, :],
                            start=(db == 0), stop=(db == EB - 1))
                    nc.scalar.activation(
                        dst[:, eb, :], mmt, AF.Identity, bias=bias_sb[:, eb:eb + 1])

            def proj_v(xT, w_sb, c):
                """Natural projection: v[j, e] blocks, no bias (folded out)."""
                for jbh in range(2):
                    for h, (n0, n1) in enumerate(((0, 384), (384, 768))):
                        mmt = PMM.tile([128, 384], F32, tag="mm")
                        for db in range(EB):
                            nc.tensor.matmul(
                                mmt, xT[:, db, jbh * 128:(jbh + 1) * 128],
                                w_sb[:, db, n0:n1],
                                start=(db == 0), stop=(db == EB - 1))
                        if h == 0:
                            nc.vector.tensor_copy(v_sb[:, c * 2 + jbh, n0:n1], mmt)
                        else:
                            nc.scalar.copy(v_sb[:, c * 2 + jbh, n0:n1], mmt)

            qbufs = [None] * NCH

            # ---- phase AB: project key and value, then first two q chunks
            with tc.tile_pool(name="abkv", bufs=1) as ABKV:
                wk_sb = ABKV.tile([128, EB, D], F32R, tag="wk")
                wv_sb = ABKV.tile([128, EB, D], F32R, tag="wv")
                stage_idx(0)
                stage_idx(1)
                # wk in halves so the first projection chain can start on
                # the first half while the second transfers
                nc.sync.dma_start(
                    out=wk_sb[:, 0:3, :],
                    in_=wkT[0:384, :].rearrange("(db p) e -> p db e", p=128))
                nc.sync.dma_start(
                    out=wk_sb[:, 3:6, :],
                    in_=wkT[384:768, :].rearrange("(db p) e -> p db e", p=128))
                # small constants after the critical-path loads
                ones_sb = P.tile([128, 128], BF16, tag="ones")
                nc.sync.dma_start(out=ones_sb, in_=ones[:, :])
                bq_sb = vec_sb("bq", bq[:])
                bk_sb = vec_sb("bk", bk[:])
                bv_sb = vec_sb("bv", bv[:])
                bg1_sb = vec_sb("bg1", bg1a[:])
                bg2_sb = vec_sb("bg2", bg2[:])      # host passes bg2/2
                ts_sb = vec_sb("ts", ts[0, :])      # host passes ts/2

                def proj_idx(i):
                    xT = xTs.pop(i)
                    kind, c = divmod(i, NCH)
                    if kind == 0:
                        proj_T(xT, wk_sb, kT[:, :, c * CH:(c + 1) * CH], bk_sb)
                    elif kind == 1:
                        proj_v(xT, wv_sb, c)
                    else:
                        qb = P.tile([128, EB, CH], F32R, tag="qbuf", bufs=3,
                                    name=f"qbuf{c}")
                        proj_T(xT, wq_sb, qb, bq_sb)
                        qbufs[c] = qb

                trans_chunk(0)
                trans_chunk(1)
                for i in range(2 * NCH + 2):         # key, value, q0, q1
                    proj_idx(i)
                    if i + 2 < 2 * NCH + 4:          # transposes up to q3
                        trans_chunk(i + 2)
                    if i == 3:
                        load_w(wv_sb, wvT)
                    elif i == 11:
                        load_w(wq_sb, wqT)
                    elif i == 14:
                        load_w(wg1_sb, wg1T)
                    elif i == 15:
                        load_w(wg2_sb, wg2T)

            # ---- phase C: attention + gate, software-pipelined over i-chunks
            with tc.tile_pool(name="phc", bufs=1) as CP:
                attnT = CP.tile([128, SB, CH], BF16, tag="attnT")
                attTb = CP.tile([128, EB * CH], BF16, tag="attTb")   # gate path
                attTf = CP.tile([128, EB * CH], F32R, tag="attTf")   # output path
                hT = CP.tile([128, EB * CH], BF16, tag="hT")
                g2 = CP.tile([128, EB * CH], F32, tag="g2")          # tanh(gate/2)
                g3 = CP.tile([128, EB * CH], F32, tag="g3")          # tanh(s1/2)
                av = CP.tile([128, EB * CH], F32R, tag="av")         # (att+bv)*ts/2
                gated = CP.tile([128, EB * CH], F32R, tag="gated")

                def tail_math(sl):
                    """out = (1 + tanh(.25*g2 + .25)) * av on a column slice.

                    s1 = sigmoid(gate) = .5 + .5*g2 ; s2 = sigmoid(s1)
                    s2*att*ts = (1 + tanh(.25*g2 + .25)) * (att+bv)*ts/2
                    """
                    nc.scalar.activation(
                        g3[:, sl], g2[:, sl], AF.Tanh, bias=c25_sb, scale=0.25)
                    nc.vector.scalar_tensor_tensor(
                        gated[:, sl], g3[:, sl], 1.0, av[:, sl],
                        ALU.add, ALU.mult)

                def tail_out(j):
                    """output transposes + store for iteration j."""
                    gv = gated.rearrange("p (eb i) -> p eb i", eb=EB)
                    for ib in range(2):
                        osb = CP.tile([128, D], F32, tag="osb", bufs=2,
                                      name="osb")
                        for half in range(2):
                            po = PSC.tile([128, 3, 128], F32R, tag="sc")
                            for k3 in range(3):
                                eb = half * 3 + k3
                                nc.tensor.transpose(
                                    po[:, k3, :],
                                    gv[:, eb, ib * 128:(ib + 1) * 128], ident_sb)
                            nc.vector.tensor_copy(
                                osb[:, half * 384:(half + 1) * 384], po)
                        r0 = (j * 2 + ib) * 128
                        nc.sync.dma_start(out=out[r0:r0 + 128, :], in_=osb)

                for ic in range(NCH):
                    qb = qbufs[ic]
                    last = ic == NCH - 1
                    # scores^T + exp, per j-block
                    for jb in range(SB):
                        ps = PSC.tile([128, CH], F32, tag="sc")
                        for eb in range(EB):
                            nc.tensor.matmul(
                                ps, kT[:, eb, jb * 128:(jb + 1) * 128],
                                qb[:, eb, :],
                                start=(eb == 0), stop=(eb == EB - 1))
                        nc.scalar.activation(
                            attnT[:, jb, :], ps, AF.Exp, scale=SCALE)
                    # next q chunk rides in the exp shadow
                    if ic + 2 < NCH:
                        proj_idx(2 * NCH + ic + 2)
                        if 2 * NCH + ic + 4 < len(order):
                            trans_chunk(2 * NCH + ic + 4)
                    # softmax denominator (ones^T @ exp) + reciprocal
                    dn = PDN.tile([128, CH], F32, tag="dn")
                    for jb in range(SB):
                        nc.tensor.matmul(
                            dn, ones_sb, attnT[:, jb, :],
                            start=(jb == 0), stop=(jb == SB - 1))
                    recip = CP.tile([128, CH], F32, tag="recip", bufs=2,
                                    name="recip")
                    nc.vector.reciprocal(recip, dn)
                    # attended^T; normalize on DVE; (att+bv)*ts/2 on Pool
                    for eb in range(EB):
                        pa = PMM.tile([128, CH], F32, tag="mm")
                        for jb in range(SB):
                            nc.tensor.matmul(
                                pa, v_sb[:, jb, eb * 128:(eb + 1) * 128],
                                attnT[:, jb, :],
                                start=(jb == 0), stop=(jb == SB - 1))
                        sl = slice(eb * CH, (eb + 1) * CH)
                        nc.vector.tensor_mul(attTb[:, sl], pa, recip)
                        nc.vector.tensor_mul(attTf[:, sl], pa, recip)
                        nc.gpsimd.tensor_scalar(
                            av[:, sl], attTf[:, sl], bv_sb[:, eb:eb + 1],
                            ts_sb[:, eb:eb + 1], ALU.add, ALU.mult)
                    # previous iteration's tail fills the attT-eviction wait
                    if ic > 0:
                        tail_math(slice(0, EB * CH))
                        tail_out(ic - 1)
                    # gate1: h = relu(Wg1 att + bg1')
                    for e2 in range(EB):
                        ph = PPG.tile([128, CH], F32, tag="pg")
                        for eb in range(EB):
                            nc.tensor.matmul(
                                ph, wg1_sb[:, eb, e2 * 128:(e2 + 1) * 128],
                                attTb[:, eb * CH:(eb + 1) * CH],
                                start=(eb == 0), stop=(eb == EB - 1))
                        nc.scalar.activation(
                            hT[:, e2 * CH:(e2 + 1) * CH], ph, AF.Relu,
                            bias=bg1_sb[:, e2:e2 + 1])
                    # gate2: g2 = tanh((Wg2 h + bg2)/2); fine-grained tail on
                    # the last iteration so the drain chain is short
                    for e2 in range(EB):
                        pg = PPG.tile([128, CH], F32, tag="pg")
                        for eb in range(EB):
                            nc.tensor.matmul(
                                pg, wg2_sb[:, eb, e2 * 128:(e2 + 1) * 128],
                                hT[:, eb * CH:(eb + 1) * CH],
                                start=(eb == 0), stop=(eb == EB - 1))
                        sl = slice(e2 * CH, (e2 + 1) * CH)
                        nc.scalar.activation(
                            g2[:, sl], pg, AF.Tanh,
                            bias=bg2_sb[:, e2:e2 + 1], scale=0.5)
                        if last:
                            tail_math(sl)
                if NCH:
                    tail_out(NCH - 1)

    nc.compile()
    return nc


def kernel(**inputs):
    if "nc" not in _CACHE:
        _CACHE["nc"] = _build()
    nc = _CACHE["nc"]
    q = np.ascontiguousarray(inputs["query"], dtype=np.float32)
    k = np.ascontiguousarray(inputs["key"], dtype=np.float32)
    vv = np.ascontiguousarray(inputs["value"], dtype=np.float32)
    Wg1 = np.asarray(inputs["Wg1"], np.float32)
    bv_np = np.asarray(inputs["bv"], np.float32)
    bg1a = np.asarray(inputs["bg1"], np.float32) + Wg1 @ bv_np
    shared = {
        "wqT": np.ascontiguousarray(np.asarray(inputs["Wq"], np.float32).T),
        "wkT": np.ascontiguousarray(np.asarray(inputs["Wk"], np.float32).T),
        "wvT": np.ascontiguousarray(np.asarray(inputs["Wv"], np.float32).T),
        "wg1T": np.ascontiguousarray(
            Wg1.T.astype(ml_dtypes.bfloat16)),
        "wg2T": np.ascontiguousarray(
            np.asarray(inputs["Wg2"], np.float32).T.astype(ml_dtypes.bfloat16)),
        "bq": np.ascontiguousarray(inputs["bq"], np.float32),
        "bk": np.ascontiguousarray(inputs["bk"], np.float32),
        "bv": np.ascontiguousarray(bv_np),
        "bg1a": np.ascontiguousarray(bg1a),
        "bg2": np.ascontiguousarray(
            np.asarray(inputs["bg2"], np.float32) * 0.5),
        "ts": np.ascontiguousarray(
            np.asarray(inputs["text_scale"], np.float32) * 0.5),
        "ident": np.eye(128, dtype=np.float32),
        "ones": np.ones((128, 128), dtype=ml_dtypes.bfloat16),
    }
    in_maps = [
        dict(shared, query=q[b], key=k[b], value=vv[b]) for b in range(B)
    ]
    trace = bool(inputs.get("_trace"))
    r = run_bass_kernel_spmd(nc, in_maps, list(range(B)), trace=trace)
    if trace:
        print("HW exec time:", r.exec_time_ns, "ns")
        _CACHE["last_result"] = r
    return np.stack([r.results[b]["out"] for b in range(B)], axis=0)


if __name__ == "__main__":
    pass


# revision 17
# speedup vs baseline: 1.5464x; 1.0006x over previous
"""Trainium2 Bass kernel: batched single-head attention + gate MLP.

Per-core (data-parallel over batch, 1 batch row per core):
  q = query @ Wq.T + bq ; k,v likewise
  scores = q @ k.T / sqrt(768); attn = softmax(scores)
  attended = attn @ v
  h = relu(attended @ Wg1.T + bg1); gate = sigmoid(h @ Wg2.T + bg2)
  out = sigmoid(gate) * attended * text_scale

Weights arrive pre-transposed from the host ([d, e] layout) so only the
three activation inputs are transposed on the PE. q is projected on
demand into a 3-slot SBUF ring inside the attention loop (no qT in
DRAM). v and the exp'd scores are stored bf16; the normalized attended
is evicted twice (bf16 for the gate matmul, f32r for the output path)
so the output is never quantized below f32r. The v bias is folded into
bg1 on the host plus a fused (att+bv)*(ts/2) op on the Pool engine,
legal because softmax rows sum to 1. Sigmoids use the tanh half-angle
identity so every activation lives in one act-function table set
(exp_and_others) — a single table load for the whole kernel.

Scheduling: one software-pipelined chunk loop (transpose chunk i+2
after projecting chunk i) keeps the PE fed through the projections;
in the attention loop the previous iteration's gate tail + output
transposes are emitted between attended and gate1, and the last
iteration runs a per-block tail to shorten the drain.
"""
import numpy as np
import ml_dtypes

import concourse.bass as bass
import concourse.mybir as mybir
import concourse.tile as tile
from concourse import bacc
from concourse.bass_utils import run_bass_kernel_spmd

F32 = mybir.dt.float32
F32R = mybir.dt.float32r
BF16 = mybir.dt.bfloat16
AF = mybir.ActivationFunctionType
ALU = mybir.AluOpType

B, S, D = 8, 2048, 768
EB = D // 128            # 6 feature blocks
SB = S // 128            # 16 seq blocks
CH = 256                 # seq chunk = attention i-chunk
NCH = S // CH            # 8
SCALE = 1.0 / float(np.sqrt(D))

_CACHE = {}


def _build():
    nc = bacc.Bacc(None)

    query = nc.dram_tensor("query", [S, D], F32R, kind="ExternalInput")
    key = nc.dram_tensor("key", [S, D], F32R, kind="ExternalInput")
    value = nc.dram_tensor("value", [S, D], F32R, kind="ExternalInput")
    wqT = nc.dram_tensor("wqT", [D, D], F32R, kind="ExternalInput")
    wkT = nc.dram_tensor("wkT", [D, D], F32R, kind="ExternalInput")
    wvT = nc.dram_tensor("wvT", [D, D], F32R, kind="ExternalInput")
    wg1T = nc.dram_tensor("wg1T", [D, D], BF16, kind="ExternalInput")
    wg2T = nc.dram_tensor("wg2T", [D, D], BF16, kind="ExternalInput")
    bq = nc.dram_tensor("bq", [D], F32, kind="ExternalInput")
    bk = nc.dram_tensor("bk", [D], F32, kind="ExternalInput")
    bv = nc.dram_tensor("bv", [D], F32, kind="ExternalInput")
    bg1a = nc.dram_tensor("bg1a", [D], F32, kind="ExternalInput")
    bg2 = nc.dram_tensor("bg2", [D], F32, kind="ExternalInput")
    ts = nc.dram_tensor("ts", [1, D], F32, kind="ExternalInput")
    ident = nc.dram_tensor("ident", [128, 128], F32R, kind="ExternalInput")
    ones = nc.dram_tensor("ones", [128, 128], BF16, kind="ExternalInput")
    out = nc.dram_tensor("out", [S, D], F32, kind="ExternalOutput")

    with tile.TileContext(nc) as tc:
        with tc.tile_pool(name="persist", bufs=1) as P, \
             tc.tile_pool(name="psc", bufs=3, space="PSUM") as PSC, \
             tc.tile_pool(name="pmm", bufs=2, space="PSUM") as PMM, \
             tc.tile_pool(name="pdn", bufs=1, space="PSUM") as PDN, \
             tc.tile_pool(name="ppg", bufs=2, space="PSUM") as PPG, \
             tc.tile_pool(name="abq", bufs=1) as ABQ:

            ident_sb = P.tile([128, 128], F32R, tag="ident")
            nc.sync.dma_start(out=ident_sb, in_=ident[:, :])
            c25_sb = P.tile([128, 1], F32, tag="c25")
            nc.vector.memset(c25_sb, 0.25)

            def vec_sb(name, src):                       # [D] -> [128, EB]
                t = P.tile([128, EB], F32, tag=name, name=name)
                nc.sync.dma_start(out=t, in_=src.rearrange("(b p) -> p b", p=128))
                return t

            kT = P.tile([128, EB, S], F32R, tag="kT")        # k^T [e, s]
            v_sb = P.tile([128, SB, D], BF16, tag="v")       # v [j, e]
            wg1_sb = P.tile([128, EB, D], BF16, tag="wg1")
            wg2_sb = P.tile([128, EB, D], BF16, tag="wg2")

            wq_sb = ABQ.tile([128, EB, D], F32R, tag="wq")

            def load_w(dst, wdram):
                nc.sync.dma_start(
                    out=dst, in_=wdram.rearrange("(db p) e -> p db e", p=128))

            # ---- staged input pipeline (key 0..7, value 8..15, query 16..23)
            order = [(key, c) for c in range(NCH)] + \
                    [(value, c) for c in range(NCH)] + \
                    [(query, c) for c in range(NCH)]
            xsts = {}
            xTs = {}

            def stage_idx(i):
                src, c = order[i]
                xst = ABQ.tile([128, 2, D], F32R, tag="xst", bufs=2)
                nc.sync.dma_start(
                    out=xst,
                    in_=src[c * CH:(c + 1) * CH, :].rearrange(
                        "(sb p) d -> p sb d", p=128))
                xsts[i] = xst

            def trans_chunk(i):
                """PE-transpose staged chunk i -> xT [d-part, db, s]."""
                xst = xsts.pop(i)
                if i + 2 < len(order):
                    stage_idx(i + 2)
                xT = ABQ.tile([128, EB, CH], F32R, tag="xT", bufs=2)
                n = 0
                for sb in range(2):
                    for db0 in (0, 3):
                        tp = PSC.tile([128, 3, 128], F32R, tag="sc")
                        for k3 in range(3):
                            nc.tensor.transpose(
                                tp[:, k3, :],
                                xst[:, sb, (db0 + k3) * 128:(db0 + k3 + 1) * 128],
                                ident_sb)
                        dst = xT[:, db0:db0 + 3, sb * 128:(sb + 1) * 128]
                        if n == 1:
                            nc.scalar.copy(dst, tp)
                        else:
                            nc.vector.tensor_copy(dst, tp)
                        n += 1
                xTs[i] = xT

            def proj_T(xT, w_sb, dst, bias_sb):
                """Transposed projection: dst[:, eb, :] = (W x^T + b)[e-blk, i]."""
                for eb in range(EB):
                    mmt = PMM.tile([128, CH], F32, tag="mm")
                    for db in range(EB):
                        nc.tensor.matmul(
                            mmt, w_sb[:, db, eb * 128:(eb + 1) * 128], xT[:, db, :],
                            start=(db == 0), stop=(db == EB - 1))
                    nc.scalar.activation(
                        dst[:, eb, :], mmt, AF.Identity, bias=bias_sb[:, eb:eb + 1])

            def proj_v(xT, w_sb, c):
                """Natural projection: v[j, e] blocks, no bias (folded out)."""
                for jbh in range(2):
                    for h, (n0, n1) in enumerate(((0, 384), (384, 768))):
                        mmt = PMM.tile([128, 384], F32, tag="mm")
                        for db in range(EB):
                            nc.tensor.matmul(
                                mmt, xT[:, db, jbh * 128:(jbh + 1) * 128],
                                w_sb[:, db, n0:n1],
                                start=(db == 0), stop=(db == EB - 1))
                        if h == 0:
                            nc.vector.tensor_copy(v_sb[:, c * 2 + jbh, n0:n1], mmt)
                        else:
                            nc.scalar.copy(v_sb[:, c * 2 + jbh, n0:n1], mmt)

            qbufs = [None] * NCH

            # ---- phase AB: project key and value, then first two q chunks
            with tc.tile_pool(name="abkv", bufs=1) as ABKV:
                wk_sb = ABKV.tile([128, EB, D], F32R, tag="wk")
                wv_sb = ABKV.tile([128, EB, D], F32R, tag="wv")
                stage_idx(0)
                stage_idx(1)
                # wk in halves so the first projection chain can start on
                # the first half while the second transfers
                nc.sync.dma_start(
                    out=wk_sb[:, 0:3, :],
                    in_=wkT[0:384, :].rearrange("(db p) e -> p db e", p=128))
                nc.sync.dma_start(
                    out=wk_sb[:, 3:6, :],
                    in_=wkT[384:768, :].rearrange("(db p) e -> p db e", p=128))
                # small constants after the critical-path loads
                ones_sb = P.tile([128, 128], BF16, tag="ones")
                nc.sync.dma_start(out=ones_sb, in_=ones[:, :])
                bq_sb = vec_sb("bq", bq[:])
                bk_sb = vec_sb("bk", bk[:])
                bv_sb = vec_sb("bv", bv[:])
                bg1_sb = vec_sb("bg1", bg1a[:])
                bg2_sb = vec_sb("bg2", bg2[:])      # host passes bg2/2
                ts_sb = vec_sb("ts", ts[0, :])      # host passes ts/2

                def proj_idx(i):
                    xT = xTs.pop(i)
                    kind, c = divmod(i, NCH)
                    if kind == 0:
                        proj_T(xT, wk_sb, kT[:, :, c * CH:(c + 1) * CH], bk_sb)
                    elif kind == 1:
                        proj_v(xT, wv_sb, c)
                    else:
                        qb = P.tile([128, EB, CH], F32R, tag="qbuf", bufs=3,
                                    name=f"qbuf{c}")
                        proj_T(xT, wq_sb, qb, bq_sb)
                        qbufs[c] = qb

                trans_chunk(0)
                trans_chunk(1)
                for i in range(2 * NCH + 2):         # key, value, q0, q1
                    proj_idx(i)
                    if i + 2 < 2 * NCH + 4:          # transposes up to q3
                        trans_chunk(i + 2)
                    if i == 3:
                        load_w(wv_sb, wvT)
                    elif i == 11:
                        load_w(wq_sb, wqT)
                    elif i == 14:
                        load_w(wg1_sb, wg1T)
                    elif i == 15:
                        load_w(wg2_sb, wg2T)

            # ---- phase C: attention + gate, software-pipelined over i-chunks
            with tc.tile_pool(name="phc", bufs=1) as CP:
                attnT = CP.tile([128, SB, CH], BF16, tag="attnT")
                attTb = CP.tile([128, EB * CH], BF16, tag="attTb")   # gate path
                attTf = CP.tile([128, EB * CH], F32R, tag="attTf")   # output path
                hT = CP.tile([128, EB * CH], BF16, tag="hT")
                g2 = CP.tile([128, EB * CH], F32, tag="g2")          # tanh(gate/2)
                g3 = CP.tile([128, EB * CH], F32, tag="g3")          # tanh(s1/2)
                av = CP.tile([128, EB * CH], F32R, tag="av")         # (att+bv)*ts/2
                gated = CP.tile([128, EB * CH], F32R, tag="gated")

                def tail_math(sl):
                    """out = (1 + tanh(.25*g2 + .25)) * av on a column slice.

                    s1 = sigmoid(gate) = .5 + .5*g2 ; s2 = sigmoid(s1)
                    s2*att*ts = (1 + tanh(.25*g2 + .25)) * (att+bv)*ts/2
                    """
                    nc.scalar.activation(
                        g3[:, sl], g2[:, sl], AF.Tanh, bias=c25_sb, scale=0.25)
                    nc.vector.scalar_tensor_tensor(
                        gated[:, sl], g3[:, sl], 1.0, av[:, sl],
                        ALU.add, ALU.mult)

                def tail_out(j):
                    """output transposes + store for iteration j."""
                    gv = gated.rearrange("p (eb i) -> p eb i", eb=EB)
                    for ib in range(2):
                        osb = CP.tile([128, D], F32, tag="osb", bufs=2,
                                      name="osb")
                        r0 = (j * 2 + ib) * 128
                        for half in range(2):
                            po = PSC.tile([128, 3, 128], F32R, tag="sc")
                            for k3 in range(3):
                                eb = half * 3 + k3
                                nc.tensor.transpose(
                                    po[:, k3, :],
                                    gv[:, eb, ib * 128:(ib + 1) * 128], ident_sb)
                            cs = slice(half * 384, (half + 1) * 384)
                            nc.vector.tensor_copy(osb[:, cs], po)
                            nc.sync.dma_start(
                                out=out[r0:r0 + 128, cs], in_=osb[:, cs])

                for ic in range(NCH):
                    qb = qbufs[ic]
                    last = ic == NCH - 1
                    # scores^T + exp per j-block, with the softmax-denominator
                    # chain (ones^T @ exp) interleaved one block behind so the
                    # PE never waits on the last exp
                    dn = PDN.tile([128, CH], F32, tag="dn")
                    for jb in range(SB):
                        ps = PSC.tile([128, CH], F32, tag="sc")
                        for eb in range(EB):
                            nc.tensor.matmul(
                                ps, kT[:, eb, jb * 128:(jb + 1) * 128],
                                qb[:, eb, :],
                                start=(eb == 0), stop=(eb == EB - 1))
                        nc.scalar.activation(
                            attnT[:, jb, :], ps, AF.Exp, scale=SCALE)
                        if jb >= 1:
                            nc.tensor.matmul(
                                dn, ones_sb, attnT[:, jb - 1, :],
                                start=(jb == 1), stop=False)
                    nc.tensor.matmul(
                        dn, ones_sb, attnT[:, SB - 1, :],
                        start=False, stop=True)
                    recip = CP.tile([128, CH], F32, tag="recip", bufs=2,
                                    name="recip")
                    nc.vector.reciprocal(recip, dn)
                    # next q chunk rides behind the scores
                    if ic + 2 < NCH:
                        proj_idx(2 * NCH + ic + 2)
                        if 2 * NCH + ic + 4 < len(order):
                            trans_chunk(2 * NCH + ic + 4)
                    # attended^T; normalize on DVE; (att+bv)*ts/2 on Pool
                    for eb in range(EB):
                        pa = PMM.tile([128, CH], F32, tag="mm")
                        for jb in range(SB):
                            nc.tensor.matmul(
                                pa, v_sb[:, jb, eb * 128:(eb + 1) * 128],
                                attnT[:, jb, :],
                                start=(jb == 0), stop=(jb == SB - 1))
                        sl = slice(eb * CH, (eb + 1) * CH)
                        nc.vector.tensor_mul(attTb[:, sl], pa, recip)
                        nc.vector.tensor_mul(attTf[:, sl], pa, recip)
                        nc.gpsimd.tensor_scalar(
                            av[:, sl], attTf[:, sl], bv_sb[:, eb:eb + 1],
                            ts_sb[:, eb:eb + 1], ALU.add, ALU.mult)
                    # previous iteration's tail fills the attT-eviction wait
                    if ic > 0:
                        tail_math(slice(0, EB * CH))
                        tail_out(ic - 1)
                    # gate1: h = relu(Wg1 att + bg1')
                    for e2 in range(EB):
                        ph = PPG.tile([128, CH], F32, tag="pg")
                        for eb in range(EB):
                            nc.tensor.matmul(
                                ph, wg1_sb[:, eb, e2 * 128:(e2 + 1) * 128],
                                attTb[:, eb * CH:(eb + 1) * CH],
                                start=(eb == 0), stop=(eb == EB - 1))
                        nc.scalar.activation(
                            hT[:, e2 * CH:(e2 + 1) * CH], ph, AF.Relu,
                            bias=bg1_sb[:, e2:e2 + 1])
                    # gate2: g2 = tanh((Wg2 h + bg2)/2); fine-grained tail on
                    # the last iteration so the drain chain is short
                    for e2 in range(EB):
                        pg = PPG.tile([128, CH], F32, tag="pg")
                        for eb in range(EB):
                            nc.tensor.matmul(
                                pg, wg2_sb[:, eb, e2 * 128:(e2 + 1) * 128],
                                hT[:, eb * CH:(eb + 1) * CH],
                                start=(eb == 0), stop=(eb == EB - 1))
                        sl = slice(e2 * CH, (e2 + 1) * CH)
                        nc.scalar.activation(
                            g2[:, sl], pg, AF.Tanh,
                            bias=bg2_sb[:, e2:e2 + 1], scale=0.5)
                        if last:
                            tail_math(sl)
                if NCH:
                    tail_out(NCH - 1)

    nc.compile()
    return nc


def kernel(**inputs):
    if "nc" not in _CACHE:
        _CACHE["nc"] = _build()
    nc = _CACHE["nc"]
    q = np.ascontiguousarray(inputs["query"], dtype=np.float32)
    k = np.ascontiguousarray(inputs["key"], dtype=np.float32)
    vv = np.ascontiguousarray(inputs["value"], dtype=np.float32)
    Wg1 = np.asarray(inputs["Wg1"], np.float32)
    bv_np = np.asarray(inputs["bv"], np.float32)
    bg1a = np.asarray(inputs["bg1"], np.float32) + Wg1 @ bv_np
    shared = {
        "wqT": np.ascontiguousarray(np.asarray(inputs["Wq"], np.float32).T),
        "wkT": np.ascontiguousarray(np.asarray(inputs["Wk"], np.float32).T),
        "wvT": np.ascontiguousarray(np.asarray(inputs["Wv"], np.float32).T),
        "wg1T": np.ascontiguousarray(
            Wg1.T.astype(ml_dtypes.bfloat16)),
        "wg2T": np.ascontiguousarray(
            np.asarray(inputs["Wg2"], np.float32).T.astype(ml_dtypes.bfloat16)),
        "bq": np.ascontiguousarray(inputs["bq"], np.float32),
        "bk": np.ascontiguousarray(inputs["bk"], np.float32),
        "bv": np.ascontiguousarray(bv_np),
        "bg1a": np.ascontiguousarray(bg1a),
        "bg2": np.ascontiguousarray(
            np.asarray(inputs["bg2"], np.float32) * 0.5),
        "ts": np.ascontiguousarray(
            np.asarray(inputs["text_scale"], np.float32) * 0.5),
        "ident": np.eye(128, dtype=np.float32),
        "ones": np.ones((128, 128), dtype=ml_dtypes.bfloat16),
    }
    in_maps = [
        dict(shared, query=q[b], key=k[b], value=vv[b]) for b in range(B)
    ]
    trace = bool(inputs.get("_trace"))
    r = run_bass_kernel_spmd(nc, in_maps, list(range(B)), trace=trace)
    if trace:
        print("HW exec time:", r.exec_time_ns, "ns")
        _CACHE["last_result"] = r
    return np.stack([r.results[b]["out"] for b in range(B)], axis=0)


if __name__ == "__main__":
    pass


# revision 18
# speedup vs baseline: 1.5517x; 1.0034x over previous
"""Trainium2 Bass kernel: batched single-head attention + gate MLP.

Per-core (data-parallel over batch, 1 batch row per core):
  q = query @ Wq.T + bq ; k,v likewise
  scores = q @ k.T / sqrt(768); attn = softmax(scores)
  attended = attn @ v
  h = relu(attended @ Wg1.T + bg1); gate = sigmoid(h @ Wg2.T + bg2)
  out = sigmoid(gate) * attended * text_scale

Weights arrive pre-transposed from the host ([d, e] layout) so only the
three activation inputs are transposed on the PE. q is projected on
demand into a 3-slot SBUF ring inside the attention loop (no qT in
DRAM). v and the exp'd scores are stored bf16; the normalized attended
is evicted twice (bf16 for the gate matmul, f32r for the output path)
so the output is never quantized below f32r. The v bias is folded into
bg1 on the host plus a fused (att+bv)*(ts/2) op on the Pool engine,
legal because softmax rows sum to 1. Sigmoids use the tanh half-angle
identity so every activation lives in one act-function table set
(exp_and_others) — a single table load for the whole kernel.

Scheduling: one software-pipelined chunk loop (transpose chunk i+2
after projecting chunk i) keeps the PE fed through the projections;
in the attention loop the previous iteration's gate tail + output
transposes are emitted between attended and gate1, and the last
iteration runs a per-block tail to shorten the drain.
"""
import numpy as np
import ml_dtypes

import concourse.bass as bass
import concourse.mybir as mybir
import concourse.tile as tile
from concourse import bacc
from concourse.bass_utils import run_bass_kernel_spmd

F32 = mybir.dt.float32
F32R = mybir.dt.float32r
BF16 = mybir.dt.bfloat16
AF = mybir.ActivationFunctionType
ALU = mybir.AluOpType

B, S, D = 8, 2048, 768
EB = D // 128            # 6 feature blocks
SB = S // 128            # 16 seq blocks
CH = 256                 # seq chunk = attention i-chunk
NCH = S // CH            # 8
SCALE = 1.0 / float(np.sqrt(D))

_CACHE = {}


def _build():
    nc = bacc.Bacc(None)

    query = nc.dram_tensor("query", [S, D], F32R, kind="ExternalInput")
    key = nc.dram_tensor("key", [S, D], F32R, kind="ExternalInput")
    value = nc.dram_tensor("value", [S, D], F32R, kind="ExternalInput")
    wqT = nc.dram_tensor("wqT", [D, D], F32R, kind="ExternalInput")
    wkT = nc.dram_tensor("wkT", [D, D], F32R, kind="ExternalInput")
    wvT = nc.dram_tensor("wvT", [D, D], F32R, kind="ExternalInput")
    wg1T = nc.dram_tensor("wg1T", [D, D], BF16, kind="ExternalInput")
    wg2T = nc.dram_tensor("wg2T", [D, D], BF16, kind="ExternalInput")
    bq = nc.dram_tensor("bq", [D], F32, kind="ExternalInput")
    bk = nc.dram_tensor("bk", [D], F32, kind="ExternalInput")
    bv = nc.dram_tensor("bv", [D], F32, kind="ExternalInput")
    bg1a = nc.dram_tensor("bg1a", [D], F32, kind="ExternalInput")
    bg2 = nc.dram_tensor("bg2", [D], F32, kind="ExternalInput")
    ts = nc.dram_tensor("ts", [1, D], F32, kind="ExternalInput")
    ident = nc.dram_tensor("ident", [128, 128], F32R, kind="ExternalInput")
    ones = nc.dram_tensor("ones", [128, 128], BF16, kind="ExternalInput")
    out = nc.dram_tensor("out", [S, D], F32, kind="ExternalOutput")

    with tile.TileContext(nc) as tc:
        with tc.tile_pool(name="persist", bufs=1) as P, \
             tc.tile_pool(name="psc", bufs=3, space="PSUM") as PSC, \
             tc.tile_pool(name="pmm", bufs=2, space="PSUM") as PMM, \
             tc.tile_pool(name="pdn", bufs=1, space="PSUM") as PDN, \
             tc.tile_pool(name="ppg", bufs=2, space="PSUM") as PPG, \
             tc.tile_pool(name="abq", bufs=1) as ABQ:

            ident_sb = P.tile([128, 128], F32R, tag="ident")
            nc.sync.dma_start(out=ident_sb, in_=ident[:, :])
            c25_sb = P.tile([128, 1], F32, tag="c25")
            nc.vector.memset(c25_sb, 0.25)

            def vec_sb(name, src):                       # [D] -> [128, EB]
                t = P.tile([128, EB], F32, tag=name, name=name)
                nc.sync.dma_start(out=t, in_=src.rearrange("(b p) -> p b", p=128))
                return t

            kT = P.tile([128, EB, S], F32R, tag="kT")        # k^T [e, s]
            v_sb = P.tile([128, SB, D], BF16, tag="v")       # v [j, e]
            wg1_sb = P.tile([128, EB, D], BF16, tag="wg1")
            wg2_sb = P.tile([128, EB, D], BF16, tag="wg2")

            wq_sb = ABQ.tile([128, EB, D], F32R, tag="wq")

            def load_w(dst, wdram):
                nc.sync.dma_start(
                    out=dst, in_=wdram.rearrange("(db p) e -> p db e", p=128))

            # ---- staged input pipeline (key 0..7, value 8..15, query 16..23)
            order = [(key, c) for c in range(NCH)] + \
                    [(value, c) for c in range(NCH)] + \
                    [(query, c) for c in range(NCH)]
            xsts = {}
            xTs = {}

            def stage_idx(i):
                src, c = order[i]
                xst = ABQ.tile([128, 2, D], F32R, tag="xst", bufs=2)
                nc.sync.dma_start(
                    out=xst,
                    in_=src[c * CH:(c + 1) * CH, :].rearrange(
                        "(sb p) d -> p sb d", p=128))
                xsts[i] = xst

            def trans_chunk(i):
                """PE-transpose staged chunk i -> xT [d-part, db, s]."""
                xst = xsts.pop(i)
                if i + 2 < len(order):
                    stage_idx(i + 2)
                xT = ABQ.tile([128, EB, CH], F32R, tag="xT", bufs=2)
                n = 0
                for sb in range(2):
                    for db0 in (0, 3):
                        tp = PSC.tile([128, 3, 128], F32R, tag="sc")
                        for k3 in range(3):
                            nc.tensor.transpose(
                                tp[:, k3, :],
                                xst[:, sb, (db0 + k3) * 128:(db0 + k3 + 1) * 128],
                                ident_sb)
                        dst = xT[:, db0:db0 + 3, sb * 128:(sb + 1) * 128]
                        if n == 1:
                            nc.scalar.copy(dst, tp)
                        else:
                            nc.vector.tensor_copy(dst, tp)
                        n += 1
                xTs[i] = xT

            def proj_T(xT, w_sb, dst, bias_sb):
                """Transposed projection: dst[:, eb, :] = (W x^T + b)[e-blk, i]."""
                for eb in range(EB):
                    mmt = PMM.tile([128, CH], F32, tag="mm")
                    for db in range(EB):
                        nc.tensor.matmul(
                            mmt, w_sb[:, db, eb * 128:(eb + 1) * 128], xT[:, db, :],
                            start=(db == 0), stop=(db == EB - 1))
                    nc.scalar.activation(
                        dst[:, eb, :], mmt, AF.Identity, bias=bias_sb[:, eb:eb + 1])

            def proj_v(xT, w_sb, c):
                """Natural projection: v[j, e] blocks, no bias (folded out)."""
                for jbh in range(2):
                    for h, (n0, n1) in enumerate(((0, 384), (384, 768))):
                        mmt = PMM.tile([128, 384], F32, tag="mm")
                        for db in range(EB):
                            nc.tensor.matmul(
                                mmt, xT[:, db, jbh * 128:(jbh + 1) * 128],
                                w_sb[:, db, n0:n1],
                                start=(db == 0), stop=(db == EB - 1))
                        if h == 0:
                            nc.vector.tensor_copy(v_sb[:, c * 2 + jbh, n0:n1], mmt)
                        else:
                            nc.scalar.copy(v_sb[:, c * 2 + jbh, n0:n1], mmt)

            qbufs = [None] * NCH

            # ---- phase AB: project key and value, then first two q chunks
            with tc.tile_pool(name="abkv", bufs=1) as ABKV:
                wk_sb = ABKV.tile([128, EB, D], F32R, tag="wk")
                wv_sb = ABKV.tile([128, EB, D], F32R, tag="wv")
                stage_idx(0)
                stage_idx(1)
                # wk in halves so the first projection chain can start on
                # the first half while the second transfers
                nc.sync.dma_start(
                    out=wk_sb[:, 0:3, :],
                    in_=wkT[0:384, :].rearrange("(db p) e -> p db e", p=128))
                nc.sync.dma_start(
                    out=wk_sb[:, 3:6, :],
                    in_=wkT[384:768, :].rearrange("(db p) e -> p db e", p=128))
                # small constants after the critical-path loads
                ones_sb = P.tile([128, 128], BF16, tag="ones")
                nc.sync.dma_start(out=ones_sb, in_=ones[:, :])
                bq_sb = vec_sb("bq", bq[:])
                bk_sb = vec_sb("bk", bk[:])
                bv_sb = vec_sb("bv", bv[:])
                bg1_sb = vec_sb("bg1", bg1a[:])
                bg2_sb = vec_sb("bg2", bg2[:])      # host passes bg2/2
                ts_sb = vec_sb("ts", ts[0, :])      # host passes ts/2

                def proj_idx(i):
                    xT = xTs.pop(i)
                    kind, c = divmod(i, NCH)
                    if kind == 0:
                        proj_T(xT, wk_sb, kT[:, :, c * CH:(c + 1) * CH], bk_sb)
                    elif kind == 1:
                        proj_v(xT, wv_sb, c)
                    else:
                        qb = P.tile([128, EB, CH], F32R, tag="qbuf", bufs=3,
                                    name=f"qbuf{c}")
                        proj_T(xT, wq_sb, qb, bq_sb)
                        qbufs[c] = qb

                trans_chunk(0)
                trans_chunk(1)
                for i in range(2 * NCH + 2):         # key, value, q0, q1
                    proj_idx(i)
                    if i + 2 < 2 * NCH + 4:          # transposes up to q3
                        trans_chunk(i + 2)
                    if i == 3:
                        load_w(wv_sb, wvT)
                    elif i == 11:
                        load_w(wq_sb, wqT)
                    elif i == 14:
                        load_w(wg1_sb, wg1T)
                    elif i == 15:
                        load_w(wg2_sb, wg2T)

            # ---- phase C: attention + gate, software-pipelined over i-chunks
            with tc.tile_pool(name="phc", bufs=1) as CP:
                attnT = CP.tile([128, SB, CH], BF16, tag="attnT")
                attTb = CP.tile([128, EB * CH], BF16, tag="attTb")   # gate path
                attTf = CP.tile([128, EB * CH], F32R, tag="attTf")   # output path
                hT = CP.tile([128, EB * CH], BF16, tag="hT")
                g2 = CP.tile([128, EB * CH], BF16, tag="g2")         # tanh(gate/2)
                g3 = CP.tile([128, EB * CH], F32, tag="g3")          # tanh(s1/2)
                gated = CP.tile([128, EB * CH], F32R, tag="gated")
                avs = [None] * NCH            # (att+bv)*ts/2, double-buffered
                                              # across iterations for the tail

                def tail_math(j, sl):
                    """out = (1 + tanh(.25*g2 + .25)) * av on a column slice.

                    s1 = sigmoid(gate) = .5 + .5*g2 ; s2 = sigmoid(s1)
                    s2*att*ts = (1 + tanh(.25*g2 + .25)) * (att+bv)*ts/2
                    """
                    nc.scalar.activation(
                        g3[:, sl], g2[:, sl], AF.Tanh, bias=c25_sb, scale=0.25)
                    nc.vector.scalar_tensor_tensor(
                        gated[:, sl], g3[:, sl], 1.0, avs[j][:, sl],
                        ALU.add, ALU.mult)

                def tail_out(j):
                    """output transposes + store for iteration j."""
                    gv = gated.rearrange("p (eb i) -> p eb i", eb=EB)
                    for ib in range(2):
                        osb = CP.tile([128, D], F32, tag="osb", bufs=2,
                                      name="osb")
                        r0 = (j * 2 + ib) * 128
                        for half in range(2):
                            po = PSC.tile([128, 3, 128], F32R, tag="sc")
                            for k3 in range(3):
                                eb = half * 3 + k3
                                nc.tensor.transpose(
                                    po[:, k3, :],
                                    gv[:, eb, ib * 128:(ib + 1) * 128], ident_sb)
                            cs = slice(half * 384, (half + 1) * 384)
                            nc.vector.tensor_copy(osb[:, cs], po)
                            nc.sync.dma_start(
                                out=out[r0:r0 + 128, cs], in_=osb[:, cs])

                for ic in range(NCH):
                    qb = qbufs[ic]
                    last = ic == NCH - 1
                    # scores^T + exp per j-block, with the softmax-denominator
                    # chain (ones^T @ exp) interleaved one block behind so the
                    # PE never waits on the last exp
                    dn = PDN.tile([128, CH], F32, tag="dn")
                    for jb in range(SB):
                        ps = PSC.tile([128, CH], F32, tag="sc")
                        for eb in range(EB):
                            nc.tensor.matmul(
                                ps, kT[:, eb, jb * 128:(jb + 1) * 128],
                                qb[:, eb, :],
                                start=(eb == 0), stop=(eb == EB - 1))
                        nc.scalar.activation(
                            attnT[:, jb, :], ps, AF.Exp, scale=SCALE)
                        if jb >= 1:
                            nc.tensor.matmul(
                                dn, ones_sb, attnT[:, jb - 1, :],
                                start=(jb == 1), stop=False)
                    nc.tensor.matmul(
                        dn, ones_sb, attnT[:, SB - 1, :],
                        start=False, stop=True)
                    recip = CP.tile([128, CH], F32, tag="recip", bufs=1,
                                    name="recip")
                    nc.vector.reciprocal(recip, dn)
                    # next q chunk rides behind the scores
                    if ic + 2 < NCH:
                        proj_idx(2 * NCH + ic + 2)
                        if 2 * NCH + ic + 4 < len(order):
                            trans_chunk(2 * NCH + ic + 4)
                    # attended^T; normalize on DVE; (att+bv)*ts/2 on Pool
                    av = CP.tile([128, EB * CH], F32R, tag="av", bufs=2,
                                 name=f"av{ic}")
                    avs[ic] = av
                    for eb in range(EB):
                        pa = PMM.tile([128, CH], F32, tag="mm")
                        for jb in range(SB):
                            nc.tensor.matmul(
                                pa, v_sb[:, jb, eb * 128:(eb + 1) * 128],
                                attnT[:, jb, :],
                                start=(jb == 0), stop=(jb == SB - 1))
                        sl = slice(eb * CH, (eb + 1) * CH)
                        nc.vector.tensor_mul(attTb[:, sl], pa, recip)
                        nc.vector.tensor_mul(attTf[:, sl], pa, recip)
                        nc.gpsimd.tensor_scalar(
                            av[:, sl], attTf[:, sl], bv_sb[:, eb:eb + 1],
                            ts_sb[:, eb:eb + 1], ALU.add, ALU.mult)
                    # previous iteration's tail fills the attT-eviction wait
                    if ic > 0:
                        tail_math(ic - 1, slice(0, EB * CH))
                        tail_out(ic - 1)
                    # gate1: h = relu(Wg1 att + bg1')
                    for e2 in range(EB):
                        ph = PPG.tile([128, CH], F32, tag="pg")
                        for eb in range(EB):
                            nc.tensor.matmul(
                                ph, wg1_sb[:, eb, e2 * 128:(e2 + 1) * 128],
                                attTb[:, eb * CH:(eb + 1) * CH],
                                start=(eb == 0), stop=(eb == EB - 1))
                        nc.scalar.activation(
                            hT[:, e2 * CH:(e2 + 1) * CH], ph, AF.Relu,
                            bias=bg1_sb[:, e2:e2 + 1])
                    # gate2: g2 = tanh((Wg2 h + bg2)/2); fine-grained tail on
                    # the last iteration so the drain chain is short
                    for e2 in range(EB):
                        pg = PPG.tile([128, CH], F32, tag="pg")
                        for eb in range(EB):
                            nc.tensor.matmul(
                                pg, wg2_sb[:, eb, e2 * 128:(e2 + 1) * 128],
                                hT[:, eb * CH:(eb + 1) * CH],
                                start=(eb == 0), stop=(eb == EB - 1))
                        sl = slice(e2 * CH, (e2 + 1) * CH)
                        nc.scalar.activation(
                            g2[:, sl], pg, AF.Tanh,
                            bias=bg2_sb[:, e2:e2 + 1], scale=0.5)
                        if last:
                            tail_math(ic, sl)
                if NCH:
                    tail_out(NCH - 1)

    nc.compile()
    return nc


def kernel(**inputs):
    if "nc" not in _CACHE:
        _CACHE["nc"] = _build()
    nc = _CACHE["nc"]
    q = np.ascontiguousarray(inputs["query"], dtype=np.float32)
    k = np.ascontiguousarray(inputs["key"], dtype=np.float32)
    vv = np.ascontiguousarray(inputs["value"], dtype=np.float32)
    Wg1 = np.asarray(inputs["Wg1"], np.float32)
    bv_np = np.asarray(inputs["bv"], np.float32)
    bg1a = np.asarray(inputs["bg1"], np.float32) + Wg1 @ bv_np
    shared = {
        "wqT": np.ascontiguousarray(np.asarray(inputs["Wq"], np.float32).T),
        "wkT": np.ascontiguousarray(np.asarray(inputs["Wk"], np.float32).T),
        "wvT": np.ascontiguousarray(np.asarray(inputs["Wv"], np.float32).T),
        "wg1T": np.ascontiguousarray(
            Wg1.T.astype(ml_dtypes.bfloat16)),
        "wg2T": np.ascontiguousarray(
            np.asarray(inputs["Wg2"], np.float32).T.astype(ml_dtypes.bfloat16)),
        "bq": np.ascontiguousarray(inputs["bq"], np.float32),
        "bk": np.ascontiguousarray(inputs["bk"], np.float32),
        "bv": np.ascontiguousarray(bv_np),
        "bg1a": np.ascontiguousarray(bg1a),
        "bg2": np.ascontiguousarray(
            np.asarray(inputs["bg2"], np.float32) * 0.5),
        "ts": np.ascontiguousarray(
            np.asarray(inputs["text_scale"], np.float32) * 0.5),
        "ident": np.eye(128, dtype=np.float32),
        "ones": np.ones((128, 128), dtype=ml_dtypes.bfloat16),
    }
    in_maps = [
        dict(shared, query=q[b], key=k[b], value=vv[b]) for b in range(B)
    ]
    trace = bool(inputs.get("_trace"))
    r = run_bass_kernel_spmd(nc, in_maps, list(range(B)), trace=trace)
    if trace:
        print("HW exec time:", r.exec_time_ns, "ns")
        _CACHE["last_result"] = r
    return np.stack([r.results[b]["out"] for b in range(B)], axis=0)


if __name__ == "__main__":
    pass
